# revision 13
# baseline (speedup 1.0000x reference)
"""BLOOM attention block on 8 TRN2 NeuronCores.

Tensor-parallel over heads: core c computes heads 4c..4c+3 for both batches.
Device math in bf16 with fp32 accumulation. v3 design (v2 + PE-cycle cuts):

  All weights (Wqkv-qk stripes, Wqkv-v, Wd) SBUF-resident; wqk loaded in
  kt-group chunks so the first matmuls start ~8us in (subtile deps).
  Single PSUM pool with 8 bank tags A0..A7 shared across phases so bank
  reuse is tracked per-tag (no phase-boundary PSUM drain stalls).
  Per batch half b (2048 tokens):
    phase 1a: Q^T/K^T kt-outer streaming - 8 PSUM accumulators; hs tiles
              [128,512] streamed from a host-packed layout. Q stripes get
              bias on DVE evac; K stripes are plain ACT copies (the K bias
              adds q.bk to every score of a query - constant per softmax
              row, so it cancels and is dropped).
    phase 1b: V = hs Wv, 4 PSUM accumulators per 512-token group; plain
              ACT copy evac (V bias folds into a host-side bv@Wd row).
    phase 2:  per (cq, head): causal-tiled transposed scores; NO alibi
              rank-1 matmul: the per-column stabilizer ramp cancels in
              softmax normalization, so it is only needed to keep exp in
              fp32/bf16 range - slots 0,1 (steep slopes) get it as a DVE
              add of a broadcast ramp tile; slots 2,3 skip it entirely
              (max exp arg ~ e^39, in range). Diagonal-square mask added
              by VE on [128,128]; exp on ACT straight from PSUM with
              per-partition bias alibi[k]-alibi[cq*512]; ones-reduce and
              ctx^T=V^T P restricted to the live column range.
    phase 3:  dense out_part = ctx Wd (bf16 partials), emission deferred
              one q-chunk so the softmax-normalize tail hides under
              attention.

Host: shards/casts inputs, then
  out = residual + bd + bv@Wd + sum_c out_part_c.
Self-contained: shapes hardcoded for B=2, S=2048, HID=4096, H=32, 8 cores.
"""

import math
from contextlib import ExitStack
from dataclasses import dataclass

import ml_dtypes
import numpy as np

import concourse.bacc as bacc
import concourse.mybir as mybir
import concourse.tile as tile
from concourse.bass import ts
from concourse.bass_utils import run_bass_kernel_spmd

F32 = mybir.dt.float32
BF16 = mybir.dt.bfloat16
AF = mybir.ActivationFunctionType
ALU = mybir.AluOpType
BF = ml_dtypes.bfloat16

N_CORES = 8


@dataclass(frozen=True)
class Cfg:
    B: int = 2
    S: int = 2048
    HID: int = 4096
    H_CORE: int = 4          # heads handled by this core
    HD: int = 128

    @property
    def TOKS(self):
        return self.B * self.S

    @property
    def KT(self):
        return self.HID // 128          # hid tiles (contraction)

    @property
    def QK_CT(self):
        return 2 * self.H_CORE          # q+k coltiles

    @property
    def VC(self):
        return self.H_CORE * self.HD    # v columns (<= 512)

    @property
    def NKT(self):
        return self.S // 128            # k tiles per sequence (per b)

    @property
    def MC(self):
        return self.VC // 128           # dense contraction chunks

    @property
    def CHQ(self):
        return self.S // 512            # q chunks per sequence (per b)

    @property
    def NG(self):
        return self.S // 512            # phase-1 groups per b


FULL = Cfg()

# qk stripe ct -> PSUM bank tag. Interleaved so banks A0..A3 (reused by the
# V accumulators and later by ctx/dense) are freed by alternating DVE/ACT
# evacs and are ready first.
QK_BANK = {0: 0, 4: 1, 1: 2, 5: 3, 2: 4, 6: 5, 3: 6, 7: 7}
QK_EVAC_ORDER = [0, 4, 1, 5, 2, 6, 3, 7]


def keep_tile(slot, kt, cq):
    """Alibi-decay tile skip: heads are sharded stride-8, so slot j's
    shallowest slope is 2^-(2j+2); a k-tile whose closest (k,q) pair is
    dist away contributes < e^-(slope*dist) relative - drop below e^-14."""
    dd = kt - 4 * cq
    if dd >= 0:
        return True
    slope_min = 2.0 ** (-2 * (slot + 1))
    dist = (4 * cq - kt) * 128 - 127
    return slope_min * dist <= 14.0


def input_specs(cfg: Cfg):
    c = cfg
    return {
        # host-packed hs: per (b,g): [128, KT*512] contiguous per partition
        "hs_pack": ([c.B * c.NG, 128, c.KT * 512], BF16),
        "wqkv_qk": ([c.QK_CT, 128, c.HID], BF16),
        "wqkv_v": ([128, c.KT * c.VC], BF16),
        "bias_q": ([128, c.H_CORE], F32),
        # per-column stabilizer ramp for steep slots 0,1: -slope*(0..511),
        # broadcast over partitions (bf16 rounding is a per-column factor
        # that cancels in softmax normalization)
        "rampq": ([128, 2 * 512], BF16),
        # exp bias per (b, hl, kt, cq): alibi[k] - alibi[cq*512]
        "bias_kq": ([128, c.B * c.H_CORE * c.NKT * c.CHQ], F32),
        "wd": ([c.MC * 128, c.HID], BF16),
    }


def output_specs(cfg: Cfg):
    return {"out_part": ([cfg.TOKS, cfg.HID], BF16)}


def build(ctx: ExitStack, tc, outs, ins, cfg: Cfg):
    c = cfg
    nc = tc.nc
    hs_pack = ins["hs_pack"]
    wqkv_qk, wqkv_v, wd = ins["wqkv_qk"], ins["wqkv_v"], ins["wd"]
    bias_q, rampq, bias_kq = ins["bias_q"], ins["rampq"], ins["bias_kq"]
    out_part = outs["out_part"]

    # ---- persistent SBUF ----
    persist = ctx.enter_context(tc.tile_pool(name="persist", bufs=1))
    wqk_sb = persist.tile([128, c.QK_CT, c.HID], BF16, tag="wqk")
    wv_sb = persist.tile([128, c.KT * c.VC], BF16, tag="wv")
    wd_sb = persist.tile([128, c.MC, c.HID], BF16, tag="wd")
    qkt_sb = persist.tile([128, c.QK_CT, c.S], BF16, tag="qkt")      # per-b
    v_sb = persist.tile([128, c.NKT, c.VC], BF16, tag="v")           # per-b
    bias_q_sb = persist.tile([128, c.H_CORE], F32, tag="bias_q")
    rampq_sb = persist.tile([128, 2, 512], BF16, tag="rampq")
    bias_kq_sb = persist.tile(
        [128, c.B * c.H_CORE * c.NKT * c.CHQ], F32, tag="bias_kq"
    )
    ones_col = persist.tile([128, 1], BF16, tag="ones_col")
    sqmask = persist.tile([128, 128], F32, tag="sqmask")

    # single PSUM pool: 8 bank tags shared by every phase so reuse is
    # tracked per-bank (fine-grained WAR instead of pool-boundary drains)
    psum = ctx.enter_context(tc.tile_pool(name="psum", bufs=1, space="PSUM"))

    def bank(i, shape=None, name=None):
        return psum.tile(
            shape or [128, 512], F32, tag=f"A{i}", name=name or f"A{i}"
        )

    # ---- startup DMA order: first hs chunk, tiny aux, then wqk kt-group 0.
    # Phase 1a's chunk loop has ~5.5us/chunk of DMA slack (6.9us compute vs
    # 1.4us hs traffic), so the remaining wqk kt-groups, wv and wd stream in
    # one kt-group ahead of use, doled out inside the loop.
    hs_pool = ctx.enter_context(tc.tile_pool(name="hs", bufs=2))
    hs_first = hs_pool.tile([128, 4, 512], BF16, tag="hs", name="hs_t")
    nc.sync.dma_start(out=hs_first[:], in_=hs_pack[0][:, 0:4 * 512])
    nc.sync.dma_start(out=bias_q_sb[:], in_=bias_q[:])
    nc.sync.dma_start(out=rampq_sb[:], in_=rampq[:])
    nc.sync.dma_start(out=bias_kq_sb[:], in_=bias_kq[:])
    for ct in range(c.QK_CT):
        nc.sync.dma_start(
            out=wqk_sb[:, ct, ts(0, 1024)], in_=wqkv_qk[ct][:, ts(0, 1024)]
        )
    # deferred weight loads: [chunk index in b0's 1a when to issue] -> DMAs
    wload = {}
    for kg in range(1, c.KT // 8):
        wload[2 * kg - 2] = [
            (lambda kg=kg, ct=ct: nc.sync.dma_start(
                out=wqk_sb[:, ct, ts(kg, 1024)],
                in_=wqkv_qk[ct][:, ts(kg, 1024)],
            ))
            for ct in range(c.QK_CT)
        ]
    for wc in range(8):
        wload.setdefault(6 + wc, []).append(
            lambda wc=wc: nc.sync.dma_start(
                out=wv_sb[:, ts(wc, 4 * c.VC)],
                in_=wqkv_v[:, ts(wc, 4 * c.VC)],
            )
        )
    for mc in range(c.MC):
        wload.setdefault(14 + mc, []).append(
            lambda mc=mc: nc.sync.dma_start(
                out=wd_sb[:, mc, :], in_=wd[ts(mc, 128), :]
            )
        )
    nc.gpsimd.memset(ones_col[:], 1.0)
    # keep (0) where kp <= qf, else -1e30 (mask k>q inside diagonal square)
    nc.gpsimd.memset(sqmask[:], 0.0)
    nc.gpsimd.affine_select(
        out=sqmask[:], in_=sqmask[:],
        compare_op=ALU.is_ge, fill=-1.0e30,
        base=0, pattern=[[1, 128]], channel_multiplier=-1,
    )

    for b in range(c.B):
        # ================= Phase 1a: Q^T/K^T =================
        _sid1, _ = nc.enter_named_scope(f"p1qk_b{b}", False)
        for g in range(c.NG):
            qk_ps = {
                ct: bank(QK_BANK[ct], name=f"qk_ps{ct}")
                for ct in range(c.QK_CT)
            }
            for ch in range(c.KT // 4):
                if b == 0 and g == 0 and ch == 0:
                    hs_t = hs_first
                else:
                    hs_t = hs_pool.tile(
                        [128, 4, 512], BF16, tag="hs", name="hs_t"
                    )
                    nc.sync.dma_start(
                        out=hs_t[:],
                        in_=hs_pack[b * c.NG + g][:, ts(ch, 4 * 512)],
                    )
                if b == 0:
                    for w in wload.pop(g * (c.KT // 4) + ch, []):
                        w()
                for k4 in range(4):
                    kt = ch * 4 + k4
                    for ct in range(c.QK_CT):
                        nc.tensor.matmul(
                            qk_ps[ct][:],
                            wqk_sb[:, ct, ts(kt, 128)],
                            hs_t[:, k4, :],
                            start=(kt == 0), stop=(kt == c.KT - 1),
                        )
            for ct in QK_EVAC_ORDER:
                if ct < c.H_CORE:
                    # Q stripe: add bias on DVE
                    nc.vector.tensor_scalar(
                        qkt_sb[:, ct, ts(g, 512)], qk_ps[ct][:],
                        bias_q_sb[:, ct:ct + 1], None, ALU.add,
                    )
                else:
                    # K stripe: bias cancels in softmax - plain ACT copy
                    nc.scalar.copy(qkt_sb[:, ct, ts(g, 512)], qk_ps[ct][:])
        nc.leave_named_scope(f"p1qk_b{b}", _sid1, False)

        # ================= Phase 1b: V =================
        _sid2, _ = nc.enter_named_scope(f"p1v_b{b}", False)
        for g in range(c.NG):
            v_ps = {tt: bank(tt, name=f"v_ps{tt}") for tt in range(4)}
            for ch in range(c.KT // 4):
                hs_t = hs_pool.tile([128, 4, 512], BF16, tag="hs", name="hs_t")
                nc.sync.dma_start(
                    out=hs_t[:],
                    in_=hs_pack[b * c.NG + g][:, ts(ch, 4 * 512)],
                )
                for k4 in range(4):
                    kt = ch * 4 + k4
                    for tt in range(4):
                        nc.tensor.matmul(
                            v_ps[tt][:],
                            hs_t[:, k4, ts(tt, 128)],
                            wv_sb[:, ts(kt, c.VC)],
                            start=(kt == 0), stop=(kt == c.KT - 1),
                        )
            for tt in range(4):
                nc.scalar.copy(v_sb[:, g * 4 + tt, :], v_ps[tt][:])
        nc.leave_named_scope(f"p1v_b{b}", _sid2, False)

        # ============ Phase 2+3: attention fused with dense ============
        _sid3, _ = nc.enter_named_scope(f"p23_b{b}", False)
        with (
            tc.tile_pool(name=f"a_pt{b}", bufs=1) as pt_pool,
            tc.tile_pool(name=f"a_roll{b}", bufs=1) as roll_pool,
            tc.tile_pool(name=f"a_sm{b}", bufs=1) as sm_pool,
            tc.tile_pool(name=f"d_out{b}", bufs=1) as o_pool,
        ):
            # manual bank rotation (replaces per-phase PSUM pools)
            s_rot = [4, 5, 6]
            ctx_rot = [0, 1]
            d_rot = [2, 3]
            rr = {"s": 0, "c": 0, "d": 0}

            def emit_dense_unit(cq, ctx_roll, sub, nb):
                d_ps = bank(d_rot[rr["d"] % 2], name="d_ps")
                rr["d"] += 1
                for mc in range(c.MC):
                    nc.tensor.matmul(
                        d_ps[:],
                        ctx_roll[:, mc, ts(sub, 128)],
                        wd_sb[:, mc, ts(nb, 512)],
                        start=(mc == 0), stop=(mc == c.MC - 1),
                    )
                o_sb = o_pool.tile([128, 512], BF16, tag="o_sb", bufs=2)
                if (sub + nb) % 2 == 0:
                    nc.scalar.copy(o_sb[:], d_ps[:])
                else:
                    nc.vector.tensor_scalar(
                        o_sb[:], d_ps[:], 0.0, None, ALU.add
                    )
                nc.sync.dma_start(
                    out=out_part[
                        b * c.S + cq * 512 + sub * 128:
                        b * c.S + cq * 512 + (sub + 1) * 128,
                        ts(nb, 512),
                    ],
                    in_=o_sb[:],
                )

            NB = c.HID // 512
            dense_q = []       # deferred dense units: (cq, roll, sub, nb)
            pend = []          # flat software pipeline across heads and cqs
            norm_done = [0] * c.CHQ

            def pop_one():
                e = pend.pop(0)
                e["b"]()
                if e["norm"] is not None:
                    e["norm"]()
                    norm_done[e["cq"]] += 1
                # paced drain: 2 dense units per pop keeps ~1us of PE work
                # between attention steps (covers the ACT exp latency) and
                # avoids a bursty tail; only emit units whose q-chunk is
                # fully normalized so the PE never head-of-line blocks
                for _ in range(2):
                    if dense_q and norm_done[dense_q[0][0]] == c.H_CORE:
                        emit_dense_unit(*dense_q.pop(0))

            for cq in range(c.CHQ):
                ktmax = 4 * (cq + 1)
                ctx_roll = roll_pool.tile(
                    [128, c.MC, 512], BF16, tag="ctx_roll", bufs=2
                )
                for hl in range(c.H_CORE):
                    qT = qkt_sb[:, hl, :]
                    kT = qkt_sb[:, c.H_CORE + hl, :]
                    sum_ps = bank(7, shape=[1, 512], name="sum_ps")
                    ctx_ps = bank(ctx_rot[rr["c"] % 2], name="ctx_ps")
                    rr["c"] += 1
                    kts = [kt for kt in range(ktmax) if keep_tile(hl, kt, cq)]

                    def stage_a(kt, cq=cq, hl=hl, qT=qT, kT=kT):
                        dd = kt - 4 * cq
                        qlo = max(dd, 0) * 128
                        s_ps = bank(s_rot[rr["s"] % 3], name="s_ps")
                        rr["s"] += 1
                        nc.tensor.matmul(
                            s_ps[:, qlo:512], kT[:, ts(kt, 128)],
                            qT[:, cq * 512 + qlo:cq * 512 + 512],
                            start=True, stop=True,
                        )
                        if dd >= 0:
                            nc.vector.tensor_tensor(
                                s_ps[:, qlo:qlo + 128], s_ps[:, qlo:qlo + 128],
                                sqmask[:], ALU.add,
                            )
                        if hl < 2:
                            # steep slots need the per-q stabilizer ramp for
                            # exp range (it cancels in normalization)
                            nc.vector.tensor_tensor(
                                s_ps[:, qlo:512], s_ps[:, qlo:512],
                                rampq_sb[:, hl, qlo:512], ALU.add,
                            )
                        pt = pt_pool.tile([128, 512], BF16, tag="pt", bufs=3)
                        bidx = ((b * c.H_CORE + hl) * c.NKT + kt) * c.CHQ + cq
                        nc.scalar.activation(
                            pt[:, qlo:512], s_ps[:, qlo:512], AF.Exp,
                            bias=bias_kq_sb[:, bidx:bidx + 1], scale=1.0,
                        )
                        return (qlo, pt)

                    def stage_b(kt, qlo, pt, hl=hl, kts=kts,
                                sum_ps=sum_ps, ctx_ps=ctx_ps):
                        st, sp = (kt == kts[0]), (kt == kts[-1])
                        # ctx first: its bank never waits on the normalize
                        # chain, so the PE keeps streaming if sum's does
                        nc.tensor.matmul(
                            ctx_ps[:, qlo:512],
                            v_sb[:, kt, ts(hl, 128)],
                            pt[:, qlo:512],
                            start=st, stop=sp,
                        )
                        nc.tensor.matmul(
                            sum_ps[:, qlo:512], ones_col[:], pt[:, qlo:512],
                            start=st, stop=sp,
                        )

                    def normalize(hl=hl, sum_ps=sum_ps, ctx_ps=ctx_ps,
                                  ctx_roll=ctx_roll):
                        rrow = sm_pool.tile([1, 512], F32, tag="rrow", bufs=2)
                        rrep = sm_pool.tile(
                            [128, 512], F32, tag="rrep", bufs=1
                        )
                        nc.vector.reciprocal_approx_fast(rrow[:], sum_ps[:])
                        nc.gpsimd.partition_broadcast(rrep[:], rrow[:])
                        nc.vector.tensor_tensor(
                            ctx_roll[:, hl, :], ctx_ps[:], rrep[:], ALU.mult,
                        )

                    for kt in kts:
                        qlo, pt = stage_a(kt)
                        pend.append({
                            "b": (lambda kt=kt, qlo=qlo, pt=pt,
                                  sb=stage_b: sb(kt, qlo, pt)),
                            "norm": normalize if kt == kts[-1] else None,
                            "cq": cq,
                        })
                        if len(pend) > 2:
                            pop_one()

                # queue this cq's dense for emission during the next chunk
                dense_q.extend(
                    (cq, ctx_roll, u // NB, u % NB) for u in range(4 * NB)
                )
            while pend:
                pop_one()
            while dense_q:
                emit_dense_unit(*dense_q.pop(0))
        nc.leave_named_scope(f"p23_b{b}", _sid3, False)


# ================= host side =================

def prep_shared(hidden_states, cfg):
    """hs_pack [B*NG, 128, KT*512] bf16 - shared across cores.

    pack[b*NG+g, p, kt*512+tq] = hs[b, g*512+tq, kt*128+p]
    (32 KB contiguous per partition per group -> efficient chunked DMA)
    """
    c = cfg
    hs = np.asarray(hidden_states, np.float32).reshape(c.B, c.NG, 512, c.KT, 128)
    pk = np.ascontiguousarray(hs.transpose(0, 1, 4, 3, 2))
    return pk.reshape(c.B * c.NG, 128, c.KT * 512).astype(BF)


def prep_core(alibi, Wqkv, bqkv, Wd, heads, cfg):
    """Per-core inputs for `heads` (list of H_CORE global head indices)."""
    c = cfg
    inv = 1.0 / math.sqrt(c.HD)
    Wq = np.asarray(Wqkv, np.float32).reshape(c.HID, -1, 3, c.HD)
    bq = np.asarray(bqkv, np.float32).reshape(-1, 3, c.HD)
    H = Wq.shape[1]

    # q cols pre-scaled by inv_norm; ct order: q heads then k heads
    w_q = Wq[:, heads, 0, :] * inv                      # [HID, H_CORE, HD]
    w_k = Wq[:, heads, 1, :]
    w_qk = np.concatenate([w_q, w_k], axis=1)           # [HID, QK_CT, 128]
    # -> [ct][hid_p][kt*128+col]: SBUF stripe rows are hid-within-chunk
    w_qk = w_qk.reshape(c.KT, 128, c.QK_CT, 128).transpose(2, 1, 0, 3)
    wqkv_qk = np.ascontiguousarray(w_qk.reshape(c.QK_CT, 128, c.HID)).astype(BF)

    w_v = Wq[:, heads, 2, :].reshape(c.HID, c.VC)       # [HID, VC]
    w_v = w_v.reshape(c.KT, 128, c.VC).transpose(1, 0, 2)  # [p, kt, vc]
    wqkv_v = np.ascontiguousarray(w_v.reshape(128, c.KT * c.VC)).astype(BF)

    b_q = bq[heads, 0, :] * inv                         # [H_CORE, 128]
    bias_q = np.ascontiguousarray(b_q.T).astype(np.float32)  # [128, H_CORE]

    al = np.asarray(alibi, np.float32).reshape(c.B, H, c.S)[:, heads]  # [B,HC,S]
    slope = al[:, :, 1] - al[:, :, 0]                   # [B, H_CORE]
    assert np.allclose(slope[0], slope[1]), "alibi slopes differ across batch"
    # per-q stabilizer ramp for steep slots 0,1: -slope*(0..511), broadcast
    # over partitions; bf16 (per-column rounding cancels in softmax norm)
    ramp = np.arange(512, dtype=np.float32)
    rq = (-slope[0][:2, None] * ramp[None, :]).reshape(1, -1)  # [1, 1024]
    rampq = np.ascontiguousarray(
        np.broadcast_to(rq, (128, 2 * 512))
    ).astype(BF)
    # exp bias per (b, hl, kt, cq): alibi[k] - alibi[cq*512]
    kpos = (np.arange(c.NKT)[:, None] * 128 + np.arange(128)[None, :])  # [NKT,128]
    bias_kq = (
        al[:, :, kpos.reshape(-1)].reshape(c.B, c.H_CORE, c.NKT, 128)[
            :, :, :, None, :
        ]
        - al[:, :, ::512][:, :, None, :, None]          # [B,HC,1,CHQ,1]
    )                                                    # [B,HC,NKT,CHQ,128]
    bias_kq = np.ascontiguousarray(
        bias_kq.transpose(4, 0, 1, 2, 3).reshape(128, -1)
    ).astype(np.float32)

    wd_c = np.asarray(Wd, np.float32).reshape(H, c.HD, c.HID)[heads]
    wd = np.ascontiguousarray(wd_c.reshape(c.MC * 128, c.HID)).astype(BF)

    return {
        "wqkv_qk": wqkv_qk,
        "wqkv_v": wqkv_v,
        "bias_q": bias_q,
        "rampq": rampq,
        "bias_kq": bias_kq,
        "wd": wd,
    }


def build_nc(cfg, debug=False):
    nc = bacc.Bacc("TRN2", target_bir_lowering=False, debug=debug)
    ins = {
        n: nc.dram_tensor(n, sh, dt, kind="ExternalInput").ap()
        for n, (sh, dt) in input_specs(cfg).items()
    }
    outs = {
        n: nc.dram_tensor(n, sh, dt, kind="ExternalOutput").ap()
        for n, (sh, dt) in output_specs(cfg).items()
    }
    with tile.TileContext(nc) as tc:
        with ExitStack() as es:
            build(es, tc, outs, ins, cfg)
    nc.compile()
    return nc


_NC_CACHE = {}


def _get_nc(cfg):
    if cfg not in _NC_CACHE:
        _NC_CACHE[cfg] = build_nc(cfg)
    return _NC_CACHE[cfg]


def _run(inputs, trace=False, **kwargs):
    cfg = FULL
    c = cfg
    hidden_states = np.asarray(inputs["hidden_states"], np.float32)
    residual = np.asarray(inputs["residual"], np.float32)
    alibi = np.asarray(inputs["alibi"], np.float32)
    Wqkv = np.asarray(inputs["Wqkv"], np.float32)
    bqkv = np.asarray(inputs["bqkv"], np.float32)
    Wd = np.asarray(inputs["Wd"], np.float32)
    bd = np.asarray(inputs["bd"], np.float32)

    nc = _get_nc(cfg)
    hs_pack = prep_shared(hidden_states, cfg)
    in_maps = []
    for core in range(N_CORES):
        # stride-8 head assignment: slot j gets head core + 8j, so each
        # slot's slope range is tight and the tile-skip pattern (which must
        # be uniform across the shared NEFF) stays safe and effective
        heads = [core + 8 * j for j in range(c.H_CORE)]
        m = {"hs_pack": hs_pack}
        m.update(prep_core(alibi, Wqkv, bqkv, Wd, heads, cfg))
        in_maps.append(m)

    res = run_bass_kernel_spmd(
        nc, in_maps, core_ids=list(range(N_CORES)), trace=trace, **kwargs
    )
    acc = np.zeros((c.TOKS, c.HID), np.float64)
    for r in res.results:
        acc += r["out_part"].astype(np.float64)
    # V bias folds through the dense layer: ctx+bv -> +bv@Wd on every row
    bv = np.asarray(bqkv, np.float64).reshape(-1, 3, c.HD)[:, 2, :].reshape(-1)
    bvwd = bv @ np.asarray(Wd, np.float64)
    out = (
        acc.reshape(c.B, c.S, c.HID)
        + residual.astype(np.float64) + bd + bvwd
    )
    return out.astype(np.float32), res


def kernel(**inputs):
    out, _ = _run(inputs, trace=False)
    return out


# revision 14
# speedup vs baseline: 1.0307x; 1.0307x over previous
"""BLOOM attention block on 8 TRN2 NeuronCores.

Tensor-parallel over heads: core c computes heads 4c..4c+3 for both batches.
Device math in bf16 with fp32 accumulation. v3 design (v2 + PE-cycle cuts):

  All weights (Wqkv-qk stripes, Wqkv-v, Wd) SBUF-resident; wqk loaded in
  kt-group chunks so the first matmuls start ~8us in (subtile deps).
  Single PSUM pool with 8 bank tags A0..A7 shared across phases so bank
  reuse is tracked per-tag (no phase-boundary PSUM drain stalls).
  Per batch half b (2048 tokens):
    phase 1a: Q^T/K^T kt-outer streaming - 8 PSUM accumulators; hs tiles
              [128,512] streamed from a host-packed layout. Q stripes get
              bias on DVE evac; K stripes are plain ACT copies (the K bias
              adds q.bk to every score of a query - constant per softmax
              row, so it cancels and is dropped).
    phase 1b: V = hs Wv, 4 PSUM accumulators per 512-token group; plain
              ACT copy evac (V bias folds into a host-side bv@Wd row).
    phase 2:  per (cq, head): causal-tiled transposed scores; NO alibi
              rank-1 matmul: the per-column stabilizer ramp cancels in
              softmax normalization, so it is only needed to keep exp in
              fp32/bf16 range - slots 0,1 (steep slopes) get it as a DVE
              add of a broadcast ramp tile; slots 2,3 skip it entirely
              (max exp arg ~ e^39, in range). Diagonal-square mask added
              by VE on [128,128]; exp on ACT straight from PSUM with
              per-partition bias alibi[k]-alibi[cq*512]; ones-reduce and
              ctx^T=V^T P restricted to the live column range.
    phase 3:  dense out_part = ctx Wd (bf16 partials), emission deferred
              one q-chunk so the softmax-normalize tail hides under
              attention.

Host: shards/casts inputs, then
  out = residual + bd + bv@Wd + sum_c out_part_c.
Self-contained: shapes hardcoded for B=2, S=2048, HID=4096, H=32, 8 cores.
"""

import math
from contextlib import ExitStack
from dataclasses import dataclass

import ml_dtypes
import numpy as np

import concourse.bacc as bacc
import concourse.mybir as mybir
import concourse.tile as tile
from concourse.bass import ts
from concourse.bass_utils import run_bass_kernel_spmd

F32 = mybir.dt.float32
BF16 = mybir.dt.bfloat16
AF = mybir.ActivationFunctionType
ALU = mybir.AluOpType
BF = ml_dtypes.bfloat16

N_CORES = 8


@dataclass(frozen=True)
class Cfg:
    B: int = 2
    S: int = 2048
    HID: int = 4096
    H_CORE: int = 4          # heads handled by this core
    HD: int = 128

    @property
    def TOKS(self):
        return self.B * self.S

    @property
    def KT(self):
        return self.HID // 128          # hid tiles (contraction)

    @property
    def QK_CT(self):
        return 2 * self.H_CORE          # q+k coltiles

    @property
    def VC(self):
        return self.H_CORE * self.HD    # v columns (<= 512)

    @property
    def NKT(self):
        return self.S // 128            # k tiles per sequence (per b)

    @property
    def MC(self):
        return self.VC // 128           # dense contraction chunks

    @property
    def CHQ(self):
        return self.S // 512            # q chunks per sequence (per b)

    @property
    def NG(self):
        return self.S // 512            # phase-1 groups per b


FULL = Cfg()

# qk stripe ct -> PSUM bank tag. Interleaved so banks A0..A3 (reused by the
# V accumulators and later by ctx/dense) are freed by alternating DVE/ACT
# evacs and are ready first.
QK_BANK = {0: 0, 4: 1, 1: 2, 5: 3, 2: 4, 6: 5, 3: 6, 7: 7}
QK_EVAC_ORDER = [0, 4, 1, 5, 2, 6, 3, 7]


def keep_tile(slot, kt, cq):
    """Alibi-decay tile skip: heads are sharded stride-8, so slot j's
    shallowest slope is 2^-(2j+2); a k-tile whose closest (k,q) pair is
    dist away contributes < e^-(slope*dist) relative - drop below e^-14."""
    dd = kt - 4 * cq
    if dd >= 0:
        return True
    slope_min = 2.0 ** (-2 * (slot + 1))
    dist = (4 * cq - kt) * 128 - 127
    return slope_min * dist <= 14.0


def input_specs(cfg: Cfg):
    c = cfg
    return {
        # host-packed hs: per (b,g): [128, KT*512] contiguous per partition
        "hs_pack": ([c.B * c.NG, 128, c.KT * 512], BF16),
        "wqkv_qk": ([c.QK_CT, 128, c.HID], BF16),
        "wqkv_v": ([128, c.KT * c.VC], BF16),
        "bias_q": ([128, c.H_CORE], F32),
        # per-column stabilizer ramp for steep slots 0,1: -slope*(0..511),
        # broadcast over partitions (bf16 rounding is a per-column factor
        # that cancels in softmax normalization)
        "rampq": ([128, 2 * 512], BF16),
        # exp bias per (b, hl, kt, cq): alibi[k] - alibi[cq*512]
        "bias_kq": ([128, c.B * c.H_CORE * c.NKT * c.CHQ], F32),
        "wd": ([c.MC * 128, c.HID], BF16),
    }


def output_specs(cfg: Cfg):
    return {"out_part": ([cfg.TOKS, cfg.HID], BF16)}


def build(ctx: ExitStack, tc, outs, ins, cfg: Cfg):
    c = cfg
    nc = tc.nc
    hs_pack = ins["hs_pack"]
    wqkv_qk, wqkv_v, wd = ins["wqkv_qk"], ins["wqkv_v"], ins["wd"]
    bias_q, rampq, bias_kq = ins["bias_q"], ins["rampq"], ins["bias_kq"]
    out_part = outs["out_part"]

    # ---- persistent SBUF ----
    persist = ctx.enter_context(tc.tile_pool(name="persist", bufs=1))
    wqk_sb = persist.tile([128, c.QK_CT, c.HID], BF16, tag="wqk")
    wv_sb = persist.tile([128, c.KT * c.VC], BF16, tag="wv")
    wd_sb = persist.tile([128, c.MC, c.HID], BF16, tag="wd")
    qkt_sb = persist.tile([128, c.QK_CT, c.S], BF16, tag="qkt")      # per-b
    v_sb = persist.tile([128, c.NKT, c.VC], BF16, tag="v")           # per-b
    bias_q_sb = persist.tile([128, c.H_CORE], F32, tag="bias_q")
    rampq_sb = persist.tile([128, 2, 512], BF16, tag="rampq")
    bias_kq_sb = persist.tile(
        [128, c.B * c.H_CORE * c.NKT * c.CHQ], F32, tag="bias_kq"
    )
    ones_col = persist.tile([128, 1], BF16, tag="ones_col")
    sqmask = persist.tile([128, 128], F32, tag="sqmask")

    # single PSUM pool: 8 bank tags shared by every phase so reuse is
    # tracked per-bank (fine-grained WAR instead of pool-boundary drains)
    psum = ctx.enter_context(tc.tile_pool(name="psum", bufs=1, space="PSUM"))

    def bank(i, shape=None, name=None):
        return psum.tile(
            shape or [128, 512], F32, tag=f"A{i}", name=name or f"A{i}"
        )

    # ---- startup DMA order: first hs chunk, tiny aux, then wqk kt-group 0.
    # Phase 1a's chunk loop has ~5.5us/chunk of DMA slack (6.9us compute vs
    # 1.4us hs traffic), so the remaining wqk kt-groups, wv and wd stream in
    # one kt-group ahead of use, doled out inside the loop.
    hs_pool = ctx.enter_context(tc.tile_pool(name="hs", bufs=2))
    hs_first = hs_pool.tile([128, 4, 512], BF16, tag="hs", name="hs_t")
    nc.sync.dma_start(out=hs_first[:], in_=hs_pack[0][:, 0:4 * 512])
    nc.sync.dma_start(out=bias_q_sb[:], in_=bias_q[:])
    nc.sync.dma_start(out=rampq_sb[:], in_=rampq[:])
    nc.sync.dma_start(out=bias_kq_sb[:], in_=bias_kq[:])
    for ct in range(c.QK_CT):
        nc.sync.dma_start(
            out=wqk_sb[:, ct, ts(0, 1024)], in_=wqkv_qk[ct][:, ts(0, 1024)]
        )
    # deferred weight loads: [chunk index in b0's 1a when to issue] -> DMAs
    wload = {}
    for kg in range(1, c.KT // 8):
        wload[2 * kg - 2] = [
            (lambda kg=kg, ct=ct: nc.sync.dma_start(
                out=wqk_sb[:, ct, ts(kg, 1024)],
                in_=wqkv_qk[ct][:, ts(kg, 1024)],
            ))
            for ct in range(c.QK_CT)
        ]
    for wc in range(8):
        wload.setdefault(6 + wc, []).append(
            lambda wc=wc: nc.sync.dma_start(
                out=wv_sb[:, ts(wc, 4 * c.VC)],
                in_=wqkv_v[:, ts(wc, 4 * c.VC)],
            )
        )
    for mc in range(c.MC):
        wload.setdefault(14 + mc, []).append(
            lambda mc=mc: nc.sync.dma_start(
                out=wd_sb[:, mc, :], in_=wd[ts(mc, 128), :]
            )
        )
    nc.gpsimd.memset(ones_col[:], 1.0)
    # keep (0) where kp <= qf, else -1e30 (mask k>q inside diagonal square)
    nc.gpsimd.memset(sqmask[:], 0.0)
    nc.gpsimd.affine_select(
        out=sqmask[:], in_=sqmask[:],
        compare_op=ALU.is_ge, fill=-1.0e30,
        base=0, pattern=[[1, 128]], channel_multiplier=-1,
    )

    for b in range(c.B):
        # ================= Phase 1a: Q^T/K^T =================
        _sid1, _ = nc.enter_named_scope(f"p1qk_b{b}", False)
        for g in range(c.NG):
            qk_ps = {
                ct: bank(QK_BANK[ct], name=f"qk_ps{ct}")
                for ct in range(c.QK_CT)
            }
            for ch in range(c.KT // 4):
                if b == 0 and g == 0 and ch == 0:
                    hs_t = hs_first
                else:
                    hs_t = hs_pool.tile(
                        [128, 4, 512], BF16, tag="hs", name="hs_t"
                    )
                    nc.sync.dma_start(
                        out=hs_t[:],
                        in_=hs_pack[b * c.NG + g][:, ts(ch, 4 * 512)],
                    )
                if b == 0:
                    for w in wload.pop(g * (c.KT // 4) + ch, []):
                        w()
                for k4 in range(4):
                    kt = ch * 4 + k4
                    for ct in range(c.QK_CT):
                        nc.tensor.matmul(
                            qk_ps[ct][:],
                            wqk_sb[:, ct, ts(kt, 128)],
                            hs_t[:, k4, :],
                            start=(kt == 0), stop=(kt == c.KT - 1),
                        )
            for ct in QK_EVAC_ORDER:
                if ct < c.H_CORE:
                    # Q stripe: add bias on DVE
                    nc.vector.tensor_scalar(
                        qkt_sb[:, ct, ts(g, 512)], qk_ps[ct][:],
                        bias_q_sb[:, ct:ct + 1], None, ALU.add,
                    )
                else:
                    # K stripe: bias cancels in softmax - plain ACT copy
                    nc.scalar.copy(qkt_sb[:, ct, ts(g, 512)], qk_ps[ct][:])
        nc.leave_named_scope(f"p1qk_b{b}", _sid1, False)

        # ================= Phase 1b: V =================
        _sid2, _ = nc.enter_named_scope(f"p1v_b{b}", False)
        for g in range(c.NG):
            v_ps = {tt: bank(tt, name=f"v_ps{tt}") for tt in range(4)}
            for ch in range(c.KT // 4):
                hs_t = hs_pool.tile([128, 4, 512], BF16, tag="hs", name="hs_t")
                nc.sync.dma_start(
                    out=hs_t[:],
                    in_=hs_pack[b * c.NG + g][:, ts(ch, 4 * 512)],
                )
                for k4 in range(4):
                    kt = ch * 4 + k4
                    for tt in range(4):
                        nc.tensor.matmul(
                            v_ps[tt][:],
                            hs_t[:, k4, ts(tt, 128)],
                            wv_sb[:, ts(kt, c.VC)],
                            start=(kt == 0), stop=(kt == c.KT - 1),
                        )
            for tt in range(4):
                nc.scalar.copy(v_sb[:, g * 4 + tt, :], v_ps[tt][:])
        nc.leave_named_scope(f"p1v_b{b}", _sid2, False)

        # ============ Phase 2+3: attention fused with dense ============
        _sid3, _ = nc.enter_named_scope(f"p23_b{b}", False)
        with (
            tc.tile_pool(name=f"a_pt{b}", bufs=1) as pt_pool,
            tc.tile_pool(name=f"a_roll{b}", bufs=1) as roll_pool,
            tc.tile_pool(name=f"a_sm{b}", bufs=1) as sm_pool,
            tc.tile_pool(name=f"d_out{b}", bufs=1) as o_pool,
        ):
            # manual bank rotation (replaces per-phase PSUM pools)
            s_rot = [4, 5, 6]
            ctx_rot = [0, 1]
            d_rot = [2, 3]
            rr = {"s": 0, "c": 0, "d": 0}

            def emit_dense_unit(cq, ctx_roll, sub, nb):
                d_ps = bank(d_rot[rr["d"] % 2], name="d_ps")
                rr["d"] += 1
                for mc in range(c.MC):
                    nc.tensor.matmul(
                        d_ps[:],
                        ctx_roll[:, mc, ts(sub, 128)],
                        wd_sb[:, mc, ts(nb, 512)],
                        start=(mc == 0), stop=(mc == c.MC - 1),
                    )
                o_sb = o_pool.tile([128, 512], BF16, tag="o_sb", bufs=2)
                if (sub + nb) % 2 == 0:
                    nc.scalar.copy(o_sb[:], d_ps[:])
                else:
                    nc.vector.tensor_scalar(
                        o_sb[:], d_ps[:], 0.0, None, ALU.add
                    )
                nc.sync.dma_start(
                    out=out_part[
                        b * c.S + cq * 512 + sub * 128:
                        b * c.S + cq * 512 + (sub + 1) * 128,
                        ts(nb, 512),
                    ],
                    in_=o_sb[:],
                )

            NB = c.HID // 512
            dense_q = []       # deferred dense units: (cq, roll, sub, nb)
            pend = []          # flat software pipeline across heads and cqs
            norm_done = [0] * c.CHQ

            def pop_one():
                e = pend.pop(0)
                e["b"]()
                if e["norm"] is not None:
                    e["norm"]()
                    norm_done[e["cq"]] += 1
                    # drain up to 8 deferred dense units per finished head
                    for _ in range(8):
                        if dense_q and norm_done[dense_q[0][0]] == c.H_CORE:
                            emit_dense_unit(*dense_q.pop(0))

            for cq in range(c.CHQ):
                ktmax = 4 * (cq + 1)
                ctx_roll = roll_pool.tile(
                    [128, c.MC, 512], BF16, tag="ctx_roll", bufs=2
                )
                for hl in range(c.H_CORE):
                    qT = qkt_sb[:, hl, :]
                    kT = qkt_sb[:, c.H_CORE + hl, :]
                    sum_ps = bank(7, shape=[1, 512], name="sum_ps")
                    ctx_ps = bank(ctx_rot[rr["c"] % 2], name="ctx_ps")
                    rr["c"] += 1
                    kts = [kt for kt in range(ktmax) if keep_tile(hl, kt, cq)]

                    def stage_a(kt, cq=cq, hl=hl, qT=qT, kT=kT):
                        dd = kt - 4 * cq
                        qlo = max(dd, 0) * 128
                        s_ps = bank(s_rot[rr["s"] % 3], name="s_ps")
                        rr["s"] += 1
                        nc.tensor.matmul(
                            s_ps[:, qlo:512], kT[:, ts(kt, 128)],
                            qT[:, cq * 512 + qlo:cq * 512 + 512],
                            start=True, stop=True,
                        )
                        if dd >= 0:
                            nc.vector.tensor_tensor(
                                s_ps[:, qlo:qlo + 128], s_ps[:, qlo:qlo + 128],
                                sqmask[:], ALU.add,
                            )
                        if hl < 2:
                            # steep slots need the per-q stabilizer ramp for
                            # exp range (it cancels in normalization)
                            nc.vector.tensor_tensor(
                                s_ps[:, qlo:512], s_ps[:, qlo:512],
                                rampq_sb[:, hl, qlo:512], ALU.add,
                            )
                        pt = pt_pool.tile([128, 512], BF16, tag="pt", bufs=3)
                        bidx = ((b * c.H_CORE + hl) * c.NKT + kt) * c.CHQ + cq
                        nc.scalar.activation(
                            pt[:, qlo:512], s_ps[:, qlo:512], AF.Exp,
                            bias=bias_kq_sb[:, bidx:bidx + 1], scale=1.0,
                        )
                        return (qlo, pt)

                    def stage_b(kt, qlo, pt, hl=hl, kts=kts,
                                sum_ps=sum_ps, ctx_ps=ctx_ps):
                        st, sp = (kt == kts[0]), (kt == kts[-1])
                        # ctx first: its bank never waits on the normalize
                        # chain, so the PE keeps streaming if sum's does
                        nc.tensor.matmul(
                            ctx_ps[:, qlo:512],
                            v_sb[:, kt, ts(hl, 128)],
                            pt[:, qlo:512],
                            start=st, stop=sp,
                        )
                        nc.tensor.matmul(
                            sum_ps[:, qlo:512], ones_col[:], pt[:, qlo:512],
                            start=st, stop=sp,
                        )

                    def normalize(hl=hl, sum_ps=sum_ps, ctx_ps=ctx_ps,
                                  ctx_roll=ctx_roll):
                        rrow = sm_pool.tile([1, 512], F32, tag="rrow", bufs=2)
                        rrep = sm_pool.tile(
                            [128, 512], F32, tag="rrep", bufs=1
                        )
                        nc.vector.reciprocal_approx_fast(rrow[:], sum_ps[:])
                        nc.gpsimd.partition_broadcast(rrep[:], rrow[:])
                        nc.vector.tensor_tensor(
                            ctx_roll[:, hl, :], ctx_ps[:], rrep[:], ALU.mult,
                        )

                    for kt in kts:
                        qlo, pt = stage_a(kt)
                        pend.append({
                            "b": (lambda kt=kt, qlo=qlo, pt=pt,
                                  sb=stage_b: sb(kt, qlo, pt)),
                            "norm": normalize if kt == kts[-1] else None,
                            "cq": cq,
                        })
                        if len(pend) > 2:
                            pop_one()

                # queue this cq's dense for emission during the next chunk
                dense_q.extend(
                    (cq, ctx_roll, u // NB, u % NB) for u in range(4 * NB)
                )
            while pend:
                pop_one()
            while dense_q:
                emit_dense_unit(*dense_q.pop(0))
        nc.leave_named_scope(f"p23_b{b}", _sid3, False)


# ================= host side =================

def prep_shared(hidden_states, cfg):
    """hs_pack [B*NG, 128, KT*512] bf16 - shared across cores.

    pack[b*NG+g, p, kt*512+tq] = hs[b, g*512+tq, kt*128+p]
    (32 KB contiguous per partition per group -> efficient chunked DMA)
    """
    c = cfg
    hs = np.asarray(hidden_states, np.float32).reshape(c.B, c.NG, 512, c.KT, 128)
    pk = np.ascontiguousarray(hs.transpose(0, 1, 4, 3, 2))
    return pk.reshape(c.B * c.NG, 128, c.KT * 512).astype(BF)


def prep_core(alibi, Wqkv, bqkv, Wd, heads, cfg):
    """Per-core inputs for `heads` (list of H_CORE global head indices)."""
    c = cfg
    inv = 1.0 / math.sqrt(c.HD)
    Wq = np.asarray(Wqkv, np.float32).reshape(c.HID, -1, 3, c.HD)
    bq = np.asarray(bqkv, np.float32).reshape(-1, 3, c.HD)
    H = Wq.shape[1]

    # q cols pre-scaled by inv_norm; ct order: q heads then k heads
    w_q = Wq[:, heads, 0, :] * inv                      # [HID, H_CORE, HD]
    w_k = Wq[:, heads, 1, :]
    w_qk = np.concatenate([w_q, w_k], axis=1)           # [HID, QK_CT, 128]
    # -> [ct][hid_p][kt*128+col]: SBUF stripe rows are hid-within-chunk
    w_qk = w_qk.reshape(c.KT, 128, c.QK_CT, 128).transpose(2, 1, 0, 3)
    wqkv_qk = np.ascontiguousarray(w_qk.reshape(c.QK_CT, 128, c.HID)).astype(BF)

    w_v = Wq[:, heads, 2, :].reshape(c.HID, c.VC)       # [HID, VC]
    w_v = w_v.reshape(c.KT, 128, c.VC).transpose(1, 0, 2)  # [p, kt, vc]
    wqkv_v = np.ascontiguousarray(w_v.reshape(128, c.KT * c.VC)).astype(BF)

    b_q = bq[heads, 0, :] * inv                         # [H_CORE, 128]
    bias_q = np.ascontiguousarray(b_q.T).astype(np.float32)  # [128, H_CORE]

    al = np.asarray(alibi, np.float32).reshape(c.B, H, c.S)[:, heads]  # [B,HC,S]
    slope = al[:, :, 1] - al[:, :, 0]                   # [B, H_CORE]
    assert np.allclose(slope[0], slope[1]), "alibi slopes differ across batch"
    # per-q stabilizer ramp for steep slots 0,1: -slope*(0..511), broadcast
    # over partitions; bf16 (per-column rounding cancels in softmax norm)
    ramp = np.arange(512, dtype=np.float32)
    rq = (-slope[0][:2, None] * ramp[None, :]).reshape(1, -1)  # [1, 1024]
    rampq = np.ascontiguousarray(
        np.broadcast_to(rq, (128, 2 * 512))
    ).astype(BF)
    # exp bias per (b, hl, kt, cq): alibi[k] - alibi[cq*512]
    kpos = (np.arange(c.NKT)[:, None] * 128 + np.arange(128)[None, :])  # [NKT,128]
    bias_kq = (
        al[:, :, kpos.reshape(-1)].reshape(c.B, c.H_CORE, c.NKT, 128)[
            :, :, :, None, :
        ]
        - al[:, :, ::512][:, :, None, :, None]          # [B,HC,1,CHQ,1]
    )                                                    # [B,HC,NKT,CHQ,128]
    bias_kq = np.ascontiguousarray(
        bias_kq.transpose(4, 0, 1, 2, 3).reshape(128, -1)
    ).astype(np.float32)

    wd_c = np.asarray(Wd, np.float32).reshape(H, c.HD, c.HID)[heads]
    wd = np.ascontiguousarray(wd_c.reshape(c.MC * 128, c.HID)).astype(BF)

    return {
        "wqkv_qk": wqkv_qk,
        "wqkv_v": wqkv_v,
        "bias_q": bias_q,
        "rampq": rampq,
        "bias_kq": bias_kq,
        "wd": wd,
    }


def build_nc(cfg, debug=False):
    nc = bacc.Bacc("TRN2", target_bir_lowering=False, debug=debug)
    ins = {
        n: nc.dram_tensor(n, sh, dt, kind="ExternalInput").ap()
        for n, (sh, dt) in input_specs(cfg).items()
    }
    outs = {
        n: nc.dram_tensor(n, sh, dt, kind="ExternalOutput").ap()
        for n, (sh, dt) in output_specs(cfg).items()
    }
    with tile.TileContext(nc) as tc:
        with ExitStack() as es:
            build(es, tc, outs, ins, cfg)
    nc.compile()
    return nc


_NC_CACHE = {}


def _get_nc(cfg):
    if cfg not in _NC_CACHE:
        _NC_CACHE[cfg] = build_nc(cfg)
    return _NC_CACHE[cfg]


def _run(inputs, trace=False, **kwargs):
    cfg = FULL
    c = cfg
    hidden_states = np.asarray(inputs["hidden_states"], np.float32)
    residual = np.asarray(inputs["residual"], np.float32)
    alibi = np.asarray(inputs["alibi"], np.float32)
    Wqkv = np.asarray(inputs["Wqkv"], np.float32)
    bqkv = np.asarray(inputs["bqkv"], np.float32)
    Wd = np.asarray(inputs["Wd"], np.float32)
    bd = np.asarray(inputs["bd"], np.float32)

    nc = _get_nc(cfg)
    hs_pack = prep_shared(hidden_states, cfg)
    in_maps = []
    for core in range(N_CORES):
        # stride-8 head assignment: slot j gets head core + 8j, so each
        # slot's slope range is tight and the tile-skip pattern (which must
        # be uniform across the shared NEFF) stays safe and effective
        heads = [core + 8 * j for j in range(c.H_CORE)]
        m = {"hs_pack": hs_pack}
        m.update(prep_core(alibi, Wqkv, bqkv, Wd, heads, cfg))
        in_maps.append(m)

    res = run_bass_kernel_spmd(
        nc, in_maps, core_ids=list(range(N_CORES)), trace=trace, **kwargs
    )
    acc = np.zeros((c.TOKS, c.HID), np.float64)
    for r in res.results:
        acc += r["out_part"].astype(np.float64)
    # V bias folds through the dense layer: ctx+bv -> +bv@Wd on every row
    bv = np.asarray(bqkv, np.float64).reshape(-1, 3, c.HD)[:, 2, :].reshape(-1)
    bvwd = bv @ np.asarray(Wd, np.float64)
    out = (
        acc.reshape(c.B, c.S, c.HID)
        + residual.astype(np.float64) + bd + bvwd
    )
    return out.astype(np.float32), res


def kernel(**inputs):
    out, _ = _run(inputs, trace=False)
    return out


# revision 17
# speedup vs baseline: 1.0621x; 1.0305x over previous
"""BLOOM attention block on 8 TRN2 NeuronCores.

Tensor-parallel over heads: core c computes heads 4c..4c+3 for both batches.
Device math in bf16 with fp32 accumulation. v3 design (v2 + PE-cycle cuts):

  All weights (Wqkv-qk stripes, Wqkv-v, Wd) SBUF-resident; wqk loaded in
  kt-group chunks so the first matmuls start ~8us in (subtile deps).
  Single PSUM pool with 8 bank tags A0..A7 shared across phases so bank
  reuse is tracked per-tag (no phase-boundary PSUM drain stalls).
  Per batch half b (2048 tokens):
    phase 1a: Q^T/K^T kt-outer streaming - 8 PSUM accumulators; hs tiles
              [128,512] streamed from a host-packed layout. Q stripes get
              bias on DVE evac; K stripes are plain ACT copies (the K bias
              adds q.bk to every score of a query - constant per softmax
              row, so it cancels and is dropped).
    phase 1b: V = hs Wv, 4 PSUM accumulators per 512-token group; plain
              ACT copy evac (V bias folds into a host-side bv@Wd row).
    phase 2:  per (cq, head): causal-tiled transposed scores; NO alibi
              rank-1 matmul: the per-column stabilizer ramp cancels in
              softmax normalization, so it is only needed to keep exp in
              fp32/bf16 range - slots 0,1 (steep slopes) get it as a DVE
              add of a broadcast ramp tile; slots 2,3 skip it entirely
              (max exp arg ~ e^39, in range). Diagonal-square mask added
              by VE on [128,128]; exp on ACT straight from PSUM with
              per-partition bias alibi[k]-alibi[cq*512]; ones-reduce and
              ctx^T=V^T P restricted to the live column range.
    phase 3:  dense out_part = ctx Wd (bf16 partials), emission deferred
              one q-chunk so the softmax-normalize tail hides under
              attention.

Host: shards/casts inputs, then
  out = residual + bd + bv@Wd + sum_c out_part_c.
Self-contained: shapes hardcoded for B=2, S=2048, HID=4096, H=32, 8 cores.
"""

import math
from contextlib import ExitStack
from dataclasses import dataclass

import ml_dtypes
import numpy as np

import concourse.bacc as bacc
import concourse.mybir as mybir
import concourse.tile as tile
from concourse.bass import ts
from concourse.bass_utils import run_bass_kernel_spmd

F32 = mybir.dt.float32
BF16 = mybir.dt.bfloat16
AF = mybir.ActivationFunctionType
ALU = mybir.AluOpType
BF = ml_dtypes.bfloat16

N_CORES = 8


@dataclass(frozen=True)
class Cfg:
    B: int = 2
    S: int = 2048
    HID: int = 4096
    H_CORE: int = 4          # heads handled by this core
    HD: int = 128

    @property
    def TOKS(self):
        return self.B * self.S

    @property
    def KT(self):
        return self.HID // 128          # hid tiles (contraction)

    @property
    def QK_CT(self):
        return 2 * self.H_CORE          # q+k coltiles

    @property
    def VC(self):
        return self.H_CORE * self.HD    # v columns (<= 512)

    @property
    def NKT(self):
        return self.S // 128            # k tiles per sequence (per b)

    @property
    def MC(self):
        return self.VC // 128           # dense contraction chunks

    @property
    def CHQ(self):
        return self.S // 512            # q chunks per sequence (per b)

    @property
    def NG(self):
        return self.S // 512            # phase-1 groups per b


FULL = Cfg()

# qk stripe ct -> PSUM bank tag. Interleaved so banks A0..A3 (reused by the
# V accumulators and later by ctx/dense) are freed by alternating DVE/ACT
# evacs and are ready first.
QK_BANK = {0: 0, 4: 1, 1: 2, 5: 3, 2: 4, 6: 5, 3: 6, 7: 7}
QK_EVAC_ORDER = [0, 4, 1, 5, 2, 6, 3, 7]


def keep_tile(slot, kt, cq):
    """Alibi-decay tile skip: heads are sharded stride-8, so slot j's
    shallowest slope is 2^-(2j+2); a k-tile whose closest (k,q) pair is
    dist away contributes < e^-(slope*dist) relative - drop below e^-14."""
    dd = kt - 4 * cq
    if dd >= 0:
        return True
    slope_min = 2.0 ** (-2 * (slot + 1))
    dist = (4 * cq - kt) * 128 - 127
    return slope_min * dist <= 14.0


def input_specs(cfg: Cfg):
    c = cfg
    return {
        # host-packed hs: per (b,g): [128, KT*512] contiguous per partition
        "hs_pack": ([c.B * c.NG, 128, c.KT * 512], BF16),
        "wqkv_qk": ([c.QK_CT, 128, c.HID], BF16),
        "wqkv_v": ([128, c.KT * c.VC], BF16),
        "bias_q": ([128, c.H_CORE], F32),
        # per-column stabilizer ramp for steep slots 0,1: -slope*(0..511),
        # broadcast over partitions (bf16 rounding is a per-column factor
        # that cancels in softmax normalization)
        "rampq": ([128, 2 * 512], BF16),
        # exp bias per (b, hl, kt, cq): alibi[k] - alibi[cq*512]
        "bias_kq": ([128, c.B * c.H_CORE * c.NKT * c.CHQ], F32),
        "wd": ([c.MC * 128, c.HID], BF16),
    }


def output_specs(cfg: Cfg):
    return {"out_part": ([cfg.TOKS, cfg.HID], BF16)}


def build(ctx: ExitStack, tc, outs, ins, cfg: Cfg):
    c = cfg
    nc = tc.nc
    hs_pack = ins["hs_pack"]
    wqkv_qk, wqkv_v, wd = ins["wqkv_qk"], ins["wqkv_v"], ins["wd"]
    bias_q, rampq, bias_kq = ins["bias_q"], ins["rampq"], ins["bias_kq"]
    out_part = outs["out_part"]

    # ---- persistent SBUF ----
    persist = ctx.enter_context(tc.tile_pool(name="persist", bufs=1))
    wqk_sb = persist.tile([128, c.QK_CT, c.HID], BF16, tag="wqk")
    wv_sb = persist.tile([128, c.KT * c.VC], BF16, tag="wv")
    wd_sb = persist.tile([128, c.MC, c.HID], BF16, tag="wd")
    qkt_sb = persist.tile([128, c.QK_CT, c.S], BF16, tag="qkt")      # per-b
    v_sb = persist.tile([128, c.NKT, c.VC], BF16, tag="v")           # per-b
    bias_q_sb = persist.tile([128, c.H_CORE], F32, tag="bias_q")
    rampq_sb = persist.tile([128, 2, 512], BF16, tag="rampq")
    bias_kq_sb = persist.tile(
        [128, c.B * c.H_CORE * c.NKT * c.CHQ], F32, tag="bias_kq"
    )
    ones_col = persist.tile([128, 1], BF16, tag="ones_col")
    sqmask = persist.tile([128, 128], F32, tag="sqmask")

    # single PSUM pool: 8 bank tags shared by every phase so reuse is
    # tracked per-bank (fine-grained WAR instead of pool-boundary drains)
    psum = ctx.enter_context(tc.tile_pool(name="psum", bufs=1, space="PSUM"))

    def bank(i, shape=None, name=None):
        return psum.tile(
            shape or [128, 512], F32, tag=f"A{i}", name=name or f"A{i}"
        )

    # ---- startup DMA order: first hs chunk, tiny aux, then wqk kt-group 0.
    # Phase 1a's chunk loop has ~5.5us/chunk of DMA slack (6.9us compute vs
    # 1.4us hs traffic), so the remaining wqk kt-groups, wv and wd stream in
    # one kt-group ahead of use, doled out inside the loop.
    hs_pool = ctx.enter_context(tc.tile_pool(name="hs", bufs=2))
    hs_first = hs_pool.tile([128, 4, 512], BF16, tag="hs", name="hs_t")
    nc.sync.dma_start(out=hs_first[:], in_=hs_pack[0][:, 0:4 * 512])
    for ct in range(c.QK_CT):
        nc.sync.dma_start(
            out=wqk_sb[:, ct, ts(0, 1024)], in_=wqkv_qk[ct][:, ts(0, 1024)]
        )
    nc.sync.dma_start(out=bias_q_sb[:], in_=bias_q[:])
    nc.sync.dma_start(out=rampq_sb[:], in_=rampq[:])
    nc.sync.dma_start(out=bias_kq_sb[:], in_=bias_kq[:])
    # deferred weight loads: [chunk index in b0's 1a when to issue] -> DMAs
    # (each kt-group split across two chunk slots to interleave with hs)
    wload = {}
    for kg in range(1, c.KT // 8):
        for half in range(2):
            wload.setdefault(2 * kg - 2 + half, []).extend(
                (lambda kg=kg, ct=ct: nc.sync.dma_start(
                    out=wqk_sb[:, ct, ts(kg, 1024)],
                    in_=wqkv_qk[ct][:, ts(kg, 1024)],
                ))
                for ct in range(4 * half, 4 * half + 4)
            )
    for wc in range(8):
        wload.setdefault(6 + wc, []).append(
            lambda wc=wc: nc.sync.dma_start(
                out=wv_sb[:, ts(wc, 4 * c.VC)],
                in_=wqkv_v[:, ts(wc, 4 * c.VC)],
            )
        )
    for mc in range(c.MC):
        wload.setdefault(14 + mc, []).append(
            lambda mc=mc: nc.sync.dma_start(
                out=wd_sb[:, mc, :], in_=wd[ts(mc, 128), :]
            )
        )
    nc.gpsimd.memset(ones_col[:], 1.0)
    # keep (0) where kp <= qf, else -1e30 (mask k>q inside diagonal square)
    nc.gpsimd.memset(sqmask[:], 0.0)
    nc.gpsimd.affine_select(
        out=sqmask[:], in_=sqmask[:],
        compare_op=ALU.is_ge, fill=-1.0e30,
        base=0, pattern=[[1, 128]], channel_multiplier=-1,
    )

    for b in range(c.B):
        # ================= Phase 1a: Q^T/K^T =================
        _sid1, _ = nc.enter_named_scope(f"p1qk_b{b}", False)
        for g in range(c.NG):
            qk_ps = {
                ct: bank(QK_BANK[ct], name=f"qk_ps{ct}")
                for ct in range(c.QK_CT)
            }
            for ch in range(c.KT // 4):
                if b == 0 and g == 0 and ch == 0:
                    hs_t = hs_first
                else:
                    hs_t = hs_pool.tile(
                        [128, 4, 512], BF16, tag="hs", name="hs_t"
                    )
                    nc.sync.dma_start(
                        out=hs_t[:],
                        in_=hs_pack[b * c.NG + g][:, ts(ch, 4 * 512)],
                    )
                if b == 0:
                    for w in wload.pop(g * (c.KT // 4) + ch, []):
                        w()
                for k4 in range(4):
                    kt = ch * 4 + k4
                    for ct in range(c.QK_CT):
                        nc.tensor.matmul(
                            qk_ps[ct][:],
                            wqk_sb[:, ct, ts(kt, 128)],
                            hs_t[:, k4, :],
                            start=(kt == 0), stop=(kt == c.KT - 1),
                        )
            for ct in QK_EVAC_ORDER:
                if ct < c.H_CORE:
                    # Q stripe: add bias on DVE
                    nc.vector.tensor_scalar(
                        qkt_sb[:, ct, ts(g, 512)], qk_ps[ct][:],
                        bias_q_sb[:, ct:ct + 1], None, ALU.add,
                    )
                else:
                    # K stripe: bias cancels in softmax - plain ACT copy
                    nc.scalar.copy(qkt_sb[:, ct, ts(g, 512)], qk_ps[ct][:])
        nc.leave_named_scope(f"p1qk_b{b}", _sid1, False)

        # ================= Phase 1b: V =================
        _sid2, _ = nc.enter_named_scope(f"p1v_b{b}", False)
        for g in range(c.NG):
            v_ps = {tt: bank(tt, name=f"v_ps{tt}") for tt in range(4)}
            for ch in range(c.KT // 4):
                hs_t = hs_pool.tile([128, 4, 512], BF16, tag="hs", name="hs_t")
                nc.sync.dma_start(
                    out=hs_t[:],
                    in_=hs_pack[b * c.NG + g][:, ts(ch, 4 * 512)],
                )
                for k4 in range(4):
                    kt = ch * 4 + k4
                    for tt in range(4):
                        nc.tensor.matmul(
                            v_ps[tt][:],
                            hs_t[:, k4, ts(tt, 128)],
                            wv_sb[:, ts(kt, c.VC)],
                            start=(kt == 0), stop=(kt == c.KT - 1),
                        )
            for tt in range(4):
                nc.scalar.copy(v_sb[:, g * 4 + tt, :], v_ps[tt][:])
        nc.leave_named_scope(f"p1v_b{b}", _sid2, False)

        # ============ Phase 2+3: attention fused with dense ============
        _sid3, _ = nc.enter_named_scope(f"p23_b{b}", False)
        with (
            tc.tile_pool(name=f"a_pt{b}", bufs=1) as pt_pool,
            tc.tile_pool(name=f"a_roll{b}", bufs=1) as roll_pool,
            tc.tile_pool(name=f"a_sm{b}", bufs=1) as sm_pool,
            tc.tile_pool(name=f"d_out{b}", bufs=1) as o_pool,
        ):
            # manual bank rotation (replaces per-phase PSUM pools)
            s_rot = [4, 5, 6]
            ctx_rot = [0, 1]
            d_rot = [2, 3]
            rr = {"s": 0, "c": 0, "d": 0}

            def emit_dense_unit(cq, ctx_roll, sub, nb):
                d_ps = bank(d_rot[rr["d"] % 2], name="d_ps")
                rr["d"] += 1
                for mc in range(c.MC):
                    nc.tensor.matmul(
                        d_ps[:],
                        ctx_roll[:, mc, ts(sub, 128)],
                        wd_sb[:, mc, ts(nb, 512)],
                        start=(mc == 0), stop=(mc == c.MC - 1),
                    )
                o_sb = o_pool.tile([128, 512], BF16, tag="o_sb", bufs=4)
                if (sub + nb) % 2 == 0:
                    nc.scalar.copy(o_sb[:], d_ps[:])
                else:
                    nc.vector.tensor_scalar(
                        o_sb[:], d_ps[:], 0.0, None, ALU.add
                    )
                nc.sync.dma_start(
                    out=out_part[
                        b * c.S + cq * 512 + sub * 128:
                        b * c.S + cq * 512 + (sub + 1) * 128,
                        ts(nb, 512),
                    ],
                    in_=o_sb[:],
                )

            NB = c.HID // 512
            dense_q = []       # deferred dense units: (cq, roll, sub, nb)
            pend = []          # flat software pipeline across heads and cqs
            norm_done = [0] * c.CHQ

            def pop_one():
                e = pend.pop(0)
                e["b"]()
                if e["norm"] is not None:
                    e["norm"]()
                    norm_done[e["cq"]] += 1
                    # drain up to 8 deferred dense units per finished head
                    for _ in range(8):
                        if dense_q and norm_done[dense_q[0][0]] == c.H_CORE:
                            emit_dense_unit(*dense_q.pop(0))

            for cq in range(c.CHQ):
                ktmax = 4 * (cq + 1)
                ctx_roll = roll_pool.tile(
                    [128, c.MC, 512], BF16, tag="ctx_roll", bufs=2
                )
                for hl in range(c.H_CORE):
                    qT = qkt_sb[:, hl, :]
                    kT = qkt_sb[:, c.H_CORE + hl, :]
                    sum_ps = bank(7, shape=[1, 512], name="sum_ps")
                    ctx_ps = bank(ctx_rot[rr["c"] % 2], name="ctx_ps")
                    rr["c"] += 1
                    kts = [kt for kt in range(ktmax) if keep_tile(hl, kt, cq)]

                    def stage_a(kt, cq=cq, hl=hl, qT=qT, kT=kT):
                        dd = kt - 4 * cq
                        qlo = max(dd, 0) * 128
                        s_ps = bank(s_rot[rr["s"] % 3], name="s_ps")
                        rr["s"] += 1
                        nc.tensor.matmul(
                            s_ps[:, qlo:512], kT[:, ts(kt, 128)],
                            qT[:, cq * 512 + qlo:cq * 512 + 512],
                            start=True, stop=True,
                        )
                        if dd >= 0:
                            nc.vector.tensor_tensor(
                                s_ps[:, qlo:qlo + 128], s_ps[:, qlo:qlo + 128],
                                sqmask[:], ALU.add,
                            )
                        if hl < 2:
                            # steep slots need the per-q stabilizer ramp for
                            # exp range (it cancels in normalization)
                            nc.vector.tensor_tensor(
                                s_ps[:, qlo:512], s_ps[:, qlo:512],
                                rampq_sb[:, hl, qlo:512], ALU.add,
                            )
                        pt = pt_pool.tile([128, 512], BF16, tag="pt", bufs=3)
                        bidx = ((b * c.H_CORE + hl) * c.NKT + kt) * c.CHQ + cq
                        nc.scalar.activation(
                            pt[:, qlo:512], s_ps[:, qlo:512], AF.Exp,
                            bias=bias_kq_sb[:, bidx:bidx + 1], scale=1.0,
                        )
                        return (qlo, pt)

                    def stage_b(kt, qlo, pt, hl=hl, kts=kts,
                                sum_ps=sum_ps, ctx_ps=ctx_ps):
                        st, sp = (kt == kts[0]), (kt == kts[-1])
                        # ctx first: its bank never waits on the normalize
                        # chain, so the PE keeps streaming if sum's does
                        nc.tensor.matmul(
                            ctx_ps[:, qlo:512],
                            v_sb[:, kt, ts(hl, 128)],
                            pt[:, qlo:512],
                            start=st, stop=sp,
                        )
                        nc.tensor.matmul(
                            sum_ps[:, qlo:512], ones_col[:], pt[:, qlo:512],
                            start=st, stop=sp,
                        )

                    def normalize(hl=hl, sum_ps=sum_ps, ctx_ps=ctx_ps,
                                  ctx_roll=ctx_roll):
                        rrow = sm_pool.tile([1, 512], F32, tag="rrow", bufs=1)
                        rrep = sm_pool.tile(
                            [128, 512], F32, tag="rrep", bufs=1
                        )
                        nc.vector.reciprocal_approx_fast(rrow[:], sum_ps[:])
                        nc.gpsimd.partition_broadcast(rrep[:], rrow[:])
                        nc.vector.tensor_tensor(
                            ctx_roll[:, hl, :], ctx_ps[:], rrep[:], ALU.mult,
                        )

                    for kt in kts:
                        qlo, pt = stage_a(kt)
                        pend.append({
                            "b": (lambda kt=kt, qlo=qlo, pt=pt,
                                  sb=stage_b: sb(kt, qlo, pt)),
                            "norm": normalize if kt == kts[-1] else None,
                            "cq": cq,
                        })
                        if len(pend) > 2:
                            pop_one()

                # queue this cq's dense for emission during the next chunk
                dense_q.extend(
                    (cq, ctx_roll, u // NB, u % NB) for u in range(4 * NB)
                )
            while pend:
                pop_one()
            while dense_q:
                emit_dense_unit(*dense_q.pop(0))
        nc.leave_named_scope(f"p23_b{b}", _sid3, False)


# ================= host side =================

def prep_shared(hidden_states, cfg):
    """hs_pack [B*NG, 128, KT*512] bf16 - shared across cores.

    pack[b*NG+g, p, kt*512+tq] = hs[b, g*512+tq, kt*128+p]
    (32 KB contiguous per partition per group -> efficient chunked DMA)
    """
    c = cfg
    hs = np.asarray(hidden_states, np.float32).reshape(c.B, c.NG, 512, c.KT, 128)
    pk = np.ascontiguousarray(hs.transpose(0, 1, 4, 3, 2))
    return pk.reshape(c.B * c.NG, 128, c.KT * 512).astype(BF)


def prep_core(alibi, Wqkv, bqkv, Wd, heads, cfg):
    """Per-core inputs for `heads` (list of H_CORE global head indices)."""
    c = cfg
    inv = 1.0 / math.sqrt(c.HD)
    Wq = np.asarray(Wqkv, np.float32).reshape(c.HID, -1, 3, c.HD)
    bq = np.asarray(bqkv, np.float32).reshape(-1, 3, c.HD)
    H = Wq.shape[1]

    # q cols pre-scaled by inv_norm; ct order: q heads then k heads
    w_q = Wq[:, heads, 0, :] * inv                      # [HID, H_CORE, HD]
    w_k = Wq[:, heads, 1, :]
    w_qk = np.concatenate([w_q, w_k], axis=1)           # [HID, QK_CT, 128]
    # -> [ct][hid_p][kt*128+col]: SBUF stripe rows are hid-within-chunk
    w_qk = w_qk.reshape(c.KT, 128, c.QK_CT, 128).transpose(2, 1, 0, 3)
    wqkv_qk = np.ascontiguousarray(w_qk.reshape(c.QK_CT, 128, c.HID)).astype(BF)

    w_v = Wq[:, heads, 2, :].reshape(c.HID, c.VC)       # [HID, VC]
    w_v = w_v.reshape(c.KT, 128, c.VC).transpose(1, 0, 2)  # [p, kt, vc]
    wqkv_v = np.ascontiguousarray(w_v.reshape(128, c.KT * c.VC)).astype(BF)

    b_q = bq[heads, 0, :] * inv                         # [H_CORE, 128]
    bias_q = np.ascontiguousarray(b_q.T).astype(np.float32)  # [128, H_CORE]

    al = np.asarray(alibi, np.float32).reshape(c.B, H, c.S)[:, heads]  # [B,HC,S]
    slope = al[:, :, 1] - al[:, :, 0]                   # [B, H_CORE]
    assert np.allclose(slope[0], slope[1]), "alibi slopes differ across batch"
    # per-q stabilizer ramp for steep slots 0,1: -slope*(0..511), broadcast
    # over partitions; bf16 (per-column rounding cancels in softmax norm)
    ramp = np.arange(512, dtype=np.float32)
    rq = (-slope[0][:2, None] * ramp[None, :]).reshape(1, -1)  # [1, 1024]
    rampq = np.ascontiguousarray(
        np.broadcast_to(rq, (128, 2 * 512))
    ).astype(BF)
    # exp bias per (b, hl, kt, cq): alibi[k] - alibi[cq*512]
    kpos = (np.arange(c.NKT)[:, None] * 128 + np.arange(128)[None, :])  # [NKT,128]
    bias_kq = (
        al[:, :, kpos.reshape(-1)].reshape(c.B, c.H_CORE, c.NKT, 128)[
            :, :, :, None, :
        ]
        - al[:, :, ::512][:, :, None, :, None]          # [B,HC,1,CHQ,1]
    )                                                    # [B,HC,NKT,CHQ,128]
    bias_kq = np.ascontiguousarray(
        bias_kq.transpose(4, 0, 1, 2, 3).reshape(128, -1)
    ).astype(np.float32)

    wd_c = np.asarray(Wd, np.float32).reshape(H, c.HD, c.HID)[heads]
    wd = np.ascontiguousarray(wd_c.reshape(c.MC * 128, c.HID)).astype(BF)

    return {
        "wqkv_qk": wqkv_qk,
        "wqkv_v": wqkv_v,
        "bias_q": bias_q,
        "rampq": rampq,
        "bias_kq": bias_kq,
        "wd": wd,
    }


def build_nc(cfg, debug=False):
    nc = bacc.Bacc("TRN2", target_bir_lowering=False, debug=debug)
    ins = {
        n: nc.dram_tensor(n, sh, dt, kind="ExternalInput").ap()
        for n, (sh, dt) in input_specs(cfg).items()
    }
    outs = {
        n: nc.dram_tensor(n, sh, dt, kind="ExternalOutput").ap()
        for n, (sh, dt) in output_specs(cfg).items()
    }
    with tile.TileContext(nc) as tc:
        with ExitStack() as es:
            build(es, tc, outs, ins, cfg)
    nc.compile()
    return nc


_NC_CACHE = {}


def _get_nc(cfg):
    if cfg not in _NC_CACHE:
        _NC_CACHE[cfg] = build_nc(cfg)
    return _NC_CACHE[cfg]


def _run(inputs, trace=False, **kwargs):
    cfg = FULL
    c = cfg
    hidden_states = np.asarray(inputs["hidden_states"], np.float32)
    residual = np.asarray(inputs["residual"], np.float32)
    alibi = np.asarray(inputs["alibi"], np.float32)
    Wqkv = np.asarray(inputs["Wqkv"], np.float32)
    bqkv = np.asarray(inputs["bqkv"], np.float32)
    Wd = np.asarray(inputs["Wd"], np.float32)
    bd = np.asarray(inputs["bd"], np.float32)

    nc = _get_nc(cfg)
    hs_pack = prep_shared(hidden_states, cfg)
    in_maps = []
    for core in range(N_CORES):
        # stride-8 head assignment: slot j gets head core + 8j, so each
        # slot's slope range is tight and the tile-skip pattern (which must
        # be uniform across the shared NEFF) stays safe and effective
        heads = [core + 8 * j for j in range(c.H_CORE)]
        m = {"hs_pack": hs_pack}
        m.update(prep_core(alibi, Wqkv, bqkv, Wd, heads, cfg))
        in_maps.append(m)

    res = run_bass_kernel_spmd(
        nc, in_maps, core_ids=list(range(N_CORES)), trace=trace, **kwargs
    )
    acc = np.zeros((c.TOKS, c.HID), np.float64)
    for r in res.results:
        acc += r["out_part"].astype(np.float64)
    # V bias folds through the dense layer: ctx+bv -> +bv@Wd on every row
    bv = np.asarray(bqkv, np.float64).reshape(-1, 3, c.HD)[:, 2, :].reshape(-1)
    bvwd = bv @ np.asarray(Wd, np.float64)
    out = (
        acc.reshape(c.B, c.S, c.HID)
        + residual.astype(np.float64) + bd + bvwd
    )
    return out.astype(np.float32), res


def kernel(**inputs):
    out, _ = _run(inputs, trace=False)
    return out


# revision 21
# speedup vs baseline: 1.1026x; 1.0381x over previous
"""BLOOM attention block on 8 TRN2 NeuronCores.

Tensor-parallel over heads: core c computes heads 4c..4c+3 for both batches.
Device math in bf16 with fp32 accumulation. v3 design (v2 + PE-cycle cuts):

  All weights (Wqkv-qk stripes, Wqkv-v, Wd) SBUF-resident; wqk loaded in
  kt-group chunks so the first matmuls start ~8us in (subtile deps).
  Single PSUM pool with 8 bank tags A0..A7 shared across phases so bank
  reuse is tracked per-tag (no phase-boundary PSUM drain stalls).
  Per batch half b (2048 tokens):
    phase 1a: Q^T/K^T kt-outer streaming - 8 PSUM accumulators; hs tiles
              [128,512] streamed from a host-packed layout. Q stripes get
              bias on DVE evac; K stripes are plain ACT copies (the K bias
              adds q.bk to every score of a query - constant per softmax
              row, so it cancels and is dropped).
    phase 1b: V = hs Wv, 4 PSUM accumulators per 512-token group; plain
              ACT copy evac (V bias folds into a host-side bv@Wd row).
    phase 2:  per (cq, head): causal-tiled transposed scores; NO alibi
              rank-1 matmul: the per-column stabilizer ramp cancels in
              softmax normalization, so it is only needed to keep exp in
              fp32/bf16 range - slots 0,1 (steep slopes) get it as a DVE
              add of a broadcast ramp tile; slots 2,3 skip it entirely
              (max exp arg ~ e^39, in range). Diagonal-square mask added
              by VE on [128,128]; exp on ACT straight from PSUM with
              per-partition bias alibi[k]-alibi[cq*512]; ones-reduce and
              ctx^T=V^T P restricted to the live column range.
    phase 3:  dense out_part = ctx Wd (bf16 partials), emission deferred
              one q-chunk so the softmax-normalize tail hides under
              attention.

Host: shards/casts inputs, then
  out = residual + bd + bv@Wd + sum_c out_part_c.
Self-contained: shapes hardcoded for B=2, S=2048, HID=4096, H=32, 8 cores.
"""

import math
from contextlib import ExitStack
from dataclasses import dataclass

import ml_dtypes
import numpy as np

import concourse.bacc as bacc
import concourse.mybir as mybir
import concourse.tile as tile
from concourse.bass import ts
from concourse.bass_utils import run_bass_kernel_spmd

F32 = mybir.dt.float32
BF16 = mybir.dt.bfloat16
AF = mybir.ActivationFunctionType
ALU = mybir.AluOpType
BF = ml_dtypes.bfloat16

N_CORES = 8


@dataclass(frozen=True)
class Cfg:
    B: int = 2
    S: int = 2048
    HID: int = 4096
    H_CORE: int = 4          # heads handled by this core
    HD: int = 128

    @property
    def TOKS(self):
        return self.B * self.S

    @property
    def KT(self):
        return self.HID // 128          # hid tiles (contraction)

    @property
    def QK_CT(self):
        return 2 * self.H_CORE          # q+k coltiles

    @property
    def VC(self):
        return self.H_CORE * self.HD    # v columns (<= 512)

    @property
    def NKT(self):
        return self.S // 128            # k tiles per sequence (per b)

    @property
    def MC(self):
        return self.VC // 128           # dense contraction chunks

    @property
    def CHQ(self):
        return self.S // 512            # q chunks per sequence (per b)

    @property
    def NG(self):
        return self.S // 512            # phase-1 groups per b


FULL = Cfg()

# qk stripe ct -> PSUM bank tag. Interleaved so banks A0..A3 (reused by the
# V accumulators and later by ctx/dense) are freed by alternating DVE/ACT
# evacs and are ready first.
QK_BANK = {0: 0, 4: 1, 1: 2, 5: 3, 2: 4, 6: 5, 3: 6, 7: 7}
QK_EVAC_ORDER = [0, 4, 1, 5, 2, 6, 3, 7]


def keep_tile(slot, kt, cq):
    """Alibi-decay tile skip: heads are sharded stride-8, so slot j's
    shallowest slope is 2^-(2j+2); a k-tile whose closest (k,q) pair is
    dist away contributes < e^-(slope*dist) relative - drop below e^-12."""
    dd = kt - 4 * cq
    if dd >= 0:
        return True
    slope_min = 2.0 ** (-2 * (slot + 1))
    dist = (4 * cq - kt) * 128 - 127
    return slope_min * dist <= 12.0


def input_specs(cfg: Cfg):
    c = cfg
    return {
        # host-packed hs: per (b,g): [128, KT*512] contiguous per partition
        "hs_pack": ([c.B * c.NG, 128, c.KT * 512], BF16),
        "wqkv_qk": ([c.QK_CT, 128, c.HID], BF16),
        "wqkv_v": ([128, c.KT * c.VC], BF16),
        "bias_q": ([128, c.H_CORE], F32),
        # per-column stabilizer ramp for steep slots 0,1: -slope*(0..511),
        # broadcast over partitions (bf16 rounding is a per-column factor
        # that cancels in softmax normalization)
        "rampq": ([128, 2 * 512], BF16),
        # exp bias per (b, hl, kt, cq): alibi[k] - alibi[cq*512]
        "bias_kq": ([128, c.B * c.H_CORE * c.NKT * c.CHQ], F32),
        "wd": ([c.MC * 128, c.HID], BF16),
    }


def output_specs(cfg: Cfg):
    return {"out_part": ([cfg.TOKS, cfg.HID], BF16)}


def build(ctx: ExitStack, tc, outs, ins, cfg: Cfg):
    c = cfg
    nc = tc.nc
    hs_pack = ins["hs_pack"]
    wqkv_qk, wqkv_v, wd = ins["wqkv_qk"], ins["wqkv_v"], ins["wd"]
    bias_q, rampq, bias_kq = ins["bias_q"], ins["rampq"], ins["bias_kq"]
    out_part = outs["out_part"]

    # ---- persistent SBUF ----
    persist = ctx.enter_context(tc.tile_pool(name="persist", bufs=1))
    wqk_sb = persist.tile([128, c.QK_CT, c.HID], BF16, tag="wqk")
    wv_sb = persist.tile([128, c.KT * c.VC], BF16, tag="wv")
    wd_sb = persist.tile([128, c.MC, c.HID], BF16, tag="wd")
    qkt_sb = persist.tile([128, c.QK_CT, c.S], BF16, tag="qkt")      # per-b
    v_sb = persist.tile([128, c.NKT, c.VC], BF16, tag="v")           # per-b
    bias_q_sb = persist.tile([128, c.H_CORE], F32, tag="bias_q")
    rampq_sb = persist.tile([128, 2, 512], BF16, tag="rampq")
    bias_kq_sb = persist.tile(
        [128, c.B * c.H_CORE * c.NKT * c.CHQ], F32, tag="bias_kq"
    )
    ones_col = persist.tile([128, 1], BF16, tag="ones_col")

    # single PSUM pool: 8 bank tags shared by every phase so reuse is
    # tracked per-bank (fine-grained WAR instead of pool-boundary drains)
    psum = ctx.enter_context(tc.tile_pool(name="psum", bufs=1, space="PSUM"))

    def bank(i, shape=None, name=None):
        return psum.tile(
            shape or [128, 512], F32, tag=f"A{i}", name=name or f"A{i}"
        )

    # ---- startup DMA order: first hs chunk, tiny aux, then wqk kt-group 0.
    # Phase 1a's chunk loop has ~5.5us/chunk of DMA slack (6.9us compute vs
    # 1.4us hs traffic), so the remaining wqk kt-groups, wv and wd stream in
    # one kt-group ahead of use, doled out inside the loop.
    hs_pool = ctx.enter_context(tc.tile_pool(name="hs", bufs=4))
    hs_first = hs_pool.tile([128, 2, 512], BF16, tag="hs", name="hs_t")
    nc.sync.dma_start(out=hs_first[:], in_=hs_pack[0][:, 0:2 * 512])
    for ct in range(c.QK_CT):
        nc.sync.dma_start(
            out=wqk_sb[:, ct, ts(0, 1024)], in_=wqkv_qk[ct][:, ts(0, 1024)]
        )
    nc.sync.dma_start(out=bias_q_sb[:], in_=bias_q[:])
    nc.sync.dma_start(out=rampq_sb[:], in_=rampq[:])
    nc.sync.dma_start(out=bias_kq_sb[:], in_=bias_kq[:])
    # deferred weight loads: [chunk index in b0's 1a when to issue] -> DMAs
    # (each kt-group split across two chunk slots to interleave with hs)
    wload = {}
    for kg in range(1, c.KT // 8):
        for half in range(2):
            wload.setdefault(4 * (kg - 1) + half, []).extend(
                (lambda kg=kg, ct=ct: nc.sync.dma_start(
                    out=wqk_sb[:, ct, ts(kg, 1024)],
                    in_=wqkv_qk[ct][:, ts(kg, 1024)],
                ))
                for ct in range(4 * half, 4 * half + 4)
            )
    for wc in range(8):
        wload.setdefault(12 + wc, []).append(
            lambda wc=wc: nc.sync.dma_start(
                out=wv_sb[:, ts(wc, 4 * c.VC)],
                in_=wqkv_v[:, ts(wc, 4 * c.VC)],
            )
        )
    for mc in range(c.MC):
        wload.setdefault(20 + mc, []).append(
            lambda mc=mc: nc.sync.dma_start(
                out=wd_sb[:, mc, :], in_=wd[ts(mc, 128), :]
            )
        )
    nc.gpsimd.memset(ones_col[:], 1.0)

    for b in range(c.B):
        # ================= Phase 1a: Q^T/K^T =================
        _sid1, _ = nc.enter_named_scope(f"p1qk_b{b}", False)
        for g in range(c.NG):
            qk_ps = {
                ct: bank(QK_BANK[ct], name=f"qk_ps{ct}")
                for ct in range(c.QK_CT)
            }
            for ch in range(c.KT // 2):
                if b == 0 and g == 0 and ch == 0:
                    hs_t = hs_first
                else:
                    hs_t = hs_pool.tile(
                        [128, 2, 512], BF16, tag="hs", name="hs_t"
                    )
                    nc.sync.dma_start(
                        out=hs_t[:],
                        in_=hs_pack[b * c.NG + g][:, ts(ch, 2 * 512)],
                    )
                if b == 0:
                    for w in wload.pop(g * (c.KT // 2) + ch, []):
                        w()
                for k4 in range(2):
                    kt = ch * 2 + k4
                    for ct in range(c.QK_CT):
                        nc.tensor.matmul(
                            qk_ps[ct][:],
                            wqk_sb[:, ct, ts(kt, 128)],
                            hs_t[:, k4, :],
                            start=(kt == 0), stop=(kt == c.KT - 1),
                        )
            for ct in QK_EVAC_ORDER:
                if ct < c.H_CORE:
                    # Q stripe: add bias on DVE
                    nc.vector.tensor_scalar(
                        qkt_sb[:, ct, ts(g, 512)], qk_ps[ct][:],
                        bias_q_sb[:, ct:ct + 1], None, ALU.add,
                    )
                else:
                    # K stripe: bias cancels in softmax - plain ACT copy
                    nc.scalar.copy(qkt_sb[:, ct, ts(g, 512)], qk_ps[ct][:])
        nc.leave_named_scope(f"p1qk_b{b}", _sid1, False)

        # ================= Phase 1b: V =================
        _sid2, _ = nc.enter_named_scope(f"p1v_b{b}", False)
        for g in range(c.NG):
            v_ps = {tt: bank(tt, name=f"v_ps{tt}") for tt in range(4)}
            for ch in range(c.KT // 2):
                hs_t = hs_pool.tile([128, 2, 512], BF16, tag="hs", name="hs_t")
                nc.sync.dma_start(
                    out=hs_t[:],
                    in_=hs_pack[b * c.NG + g][:, ts(ch, 2 * 512)],
                )
                for k4 in range(2):
                    kt = ch * 2 + k4
                    for tt in range(4):
                        nc.tensor.matmul(
                            v_ps[tt][:],
                            hs_t[:, k4, ts(tt, 128)],
                            wv_sb[:, ts(kt, c.VC)],
                            start=(kt == 0), stop=(kt == c.KT - 1),
                        )
            for tt in range(4):
                nc.scalar.copy(v_sb[:, g * 4 + tt, :], v_ps[tt][:])
        nc.leave_named_scope(f"p1v_b{b}", _sid2, False)

        # ============ Phase 2+3: attention fused with dense ============
        _sid3, _ = nc.enter_named_scope(f"p23_b{b}", False)
        with (
            tc.tile_pool(name=f"a_pt{b}", bufs=1) as pt_pool,
            tc.tile_pool(name=f"a_roll{b}", bufs=1) as roll_pool,
            tc.tile_pool(name=f"a_sm{b}", bufs=1) as sm_pool,
            tc.tile_pool(name=f"d_out{b}", bufs=1) as o_pool,
        ):
            # manual bank rotation (replaces per-phase PSUM pools)
            s_rot = [4, 5, 6]
            ctx_rot = [0, 1]
            d_rot = [2, 3]
            rr = {"s": 0, "c": 0, "d": 0}

            def emit_dense_unit(cq, ctx_roll, sub, nb):
                d_ps = bank(d_rot[rr["d"] % 2], name="d_ps")
                rr["d"] += 1
                for mc in range(c.MC):
                    nc.tensor.matmul(
                        d_ps[:],
                        ctx_roll[:, mc, ts(sub, 128)],
                        wd_sb[:, mc, ts(nb, 512)],
                        start=(mc == 0), stop=(mc == c.MC - 1),
                    )
                o_sb = o_pool.tile([128, 512], BF16, tag="o_sb", bufs=4)
                if (sub + nb) % 2 == 0:
                    nc.scalar.copy(o_sb[:], d_ps[:])
                else:
                    nc.vector.tensor_scalar(
                        o_sb[:], d_ps[:], 0.0, None, ALU.add
                    )
                nc.sync.dma_start(
                    out=out_part[
                        b * c.S + cq * 512 + sub * 128:
                        b * c.S + cq * 512 + (sub + 1) * 128,
                        ts(nb, 512),
                    ],
                    in_=o_sb[:],
                )

            NB = c.HID // 512
            dense_q = []       # deferred dense units: (cq, roll, sub, nb)
            pend = []          # flat software pipeline across heads and cqs
            norm_done = [0] * c.CHQ

            def pop_one():
                e = pend.pop(0)
                e["b"]()
                if e["norm"] is not None:
                    e["norm"]()
                    norm_done[e["cq"]] += 1
                    # drain up to 8 deferred dense units per finished head
                    for _ in range(8):
                        if dense_q and norm_done[dense_q[0][0]] == c.H_CORE:
                            emit_dense_unit(*dense_q.pop(0))

            for cq in range(c.CHQ):
                ktmax = 4 * (cq + 1)
                ctx_roll = roll_pool.tile(
                    [128, c.MC, 512], BF16, tag="ctx_roll", bufs=2
                )
                for hl in range(c.H_CORE):
                    qT = qkt_sb[:, hl, :]
                    kT = qkt_sb[:, c.H_CORE + hl, :]
                    sum_ps = bank(7, shape=[1, 512], name="sum_ps")
                    ctx_ps = bank(ctx_rot[rr["c"] % 2], name="ctx_ps")
                    rr["c"] += 1
                    kts = [kt for kt in range(ktmax) if keep_tile(hl, kt, cq)]

                    def stage_a(kt, cq=cq, hl=hl, qT=qT, kT=kT):
                        dd = kt - 4 * cq
                        qlo = max(dd, 0) * 128
                        s_ps = bank(s_rot[rr["s"] % 3], name="s_ps")
                        rr["s"] += 1
                        nc.tensor.matmul(
                            s_ps[:, qlo:512], kT[:, ts(kt, 128)],
                            qT[:, cq * 512 + qlo:cq * 512 + 512],
                            start=True, stop=True,
                        )
                        if hl < 2:
                            # steep slots need the per-q stabilizer ramp for
                            # exp range (it cancels in normalization)
                            nc.vector.tensor_tensor(
                                s_ps[:, qlo:512], s_ps[:, qlo:512],
                                rampq_sb[:, hl, qlo:512], ALU.add,
                            )
                        pt = pt_pool.tile([128, 512], BF16, tag="pt", bufs=3)
                        bidx = ((b * c.H_CORE + hl) * c.NKT + kt) * c.CHQ + cq
                        nc.scalar.activation(
                            pt[:, qlo:512], s_ps[:, qlo:512], AF.Exp,
                            bias=bias_kq_sb[:, bidx:bidx + 1], scale=1.0,
                        )
                        if dd >= 0:
                            # causal mask applied post-exp on the idle gpsimd
                            # engine: zero the k>q triangle of the diagonal
                            # square (overwrites any exp overflow exactly)
                            nc.gpsimd.affine_select(
                                out=pt[:, qlo:qlo + 128],
                                in_=pt[:, qlo:qlo + 128],
                                compare_op=ALU.is_ge, fill=0.0,
                                base=0, pattern=[[1, 128]],
                                channel_multiplier=-1,
                            )
                        return (qlo, pt)

                    def stage_b(kt, qlo, pt, hl=hl, kts=kts,
                                sum_ps=sum_ps, ctx_ps=ctx_ps):
                        st, sp = (kt == kts[0]), (kt == kts[-1])
                        # ctx first: its bank never waits on the normalize
                        # chain, so the PE keeps streaming if sum's does
                        nc.tensor.matmul(
                            ctx_ps[:, qlo:512],
                            v_sb[:, kt, ts(hl, 128)],
                            pt[:, qlo:512],
                            start=st, stop=sp,
                        )
                        nc.tensor.matmul(
                            sum_ps[:, qlo:512], ones_col[:], pt[:, qlo:512],
                            start=st, stop=sp,
                        )

                    def normalize(hl=hl, sum_ps=sum_ps, ctx_ps=ctx_ps,
                                  ctx_roll=ctx_roll):
                        rrow = sm_pool.tile([1, 512], F32, tag="rrow", bufs=1)
                        rrep = sm_pool.tile(
                            [128, 512], F32, tag="rrep", bufs=1
                        )
                        nc.vector.reciprocal_approx_fast(rrow[:], sum_ps[:])
                        nc.gpsimd.partition_broadcast(rrep[:], rrow[:])
                        nc.vector.tensor_tensor(
                            ctx_roll[:, hl, :], ctx_ps[:], rrep[:], ALU.mult,
                        )

                    for kt in kts:
                        qlo, pt = stage_a(kt)
                        pend.append({
                            "b": (lambda kt=kt, qlo=qlo, pt=pt,
                                  sb=stage_b: sb(kt, qlo, pt)),
                            "norm": normalize if kt == kts[-1] else None,
                            "cq": cq,
                        })
                        if len(pend) > 2:
                            pop_one()

                # queue this cq's dense for emission during the next chunk
                dense_q.extend(
                    (cq, ctx_roll, u // NB, u % NB) for u in range(4 * NB)
                )
            while pend:
                pop_one()
            while dense_q:
                emit_dense_unit(*dense_q.pop(0))
        nc.leave_named_scope(f"p23_b{b}", _sid3, False)


# ================= host side =================

def prep_shared(hidden_states, cfg):
    """hs_pack [B*NG, 128, KT*512] bf16 - shared across cores.

    pack[b*NG+g, p, kt*512+tq] = hs[b, g*512+tq, kt*128+p]
    (32 KB contiguous per partition per group -> efficient chunked DMA)
    """
    c = cfg
    hs = np.asarray(hidden_states, np.float32).reshape(c.B, c.NG, 512, c.KT, 128)
    pk = np.ascontiguousarray(hs.transpose(0, 1, 4, 3, 2))
    return pk.reshape(c.B * c.NG, 128, c.KT * 512).astype(BF)


def prep_core(alibi, Wqkv, bqkv, Wd, heads, cfg):
    """Per-core inputs for `heads` (list of H_CORE global head indices)."""
    c = cfg
    inv = 1.0 / math.sqrt(c.HD)
    Wq = np.asarray(Wqkv, np.float32).reshape(c.HID, -1, 3, c.HD)
    bq = np.asarray(bqkv, np.float32).reshape(-1, 3, c.HD)
    H = Wq.shape[1]

    # q cols pre-scaled by inv_norm; ct order: q heads then k heads
    w_q = Wq[:, heads, 0, :] * inv                      # [HID, H_CORE, HD]
    w_k = Wq[:, heads, 1, :]
    w_qk = np.concatenate([w_q, w_k], axis=1)           # [HID, QK_CT, 128]
    # -> [ct][hid_p][kt*128+col]: SBUF stripe rows are hid-within-chunk
    w_qk = w_qk.reshape(c.KT, 128, c.QK_CT, 128).transpose(2, 1, 0, 3)
    wqkv_qk = np.ascontiguousarray(w_qk.reshape(c.QK_CT, 128, c.HID)).astype(BF)

    w_v = Wq[:, heads, 2, :].reshape(c.HID, c.VC)       # [HID, VC]
    w_v = w_v.reshape(c.KT, 128, c.VC).transpose(1, 0, 2)  # [p, kt, vc]
    wqkv_v = np.ascontiguousarray(w_v.reshape(128, c.KT * c.VC)).astype(BF)

    b_q = bq[heads, 0, :] * inv                         # [H_CORE, 128]
    bias_q = np.ascontiguousarray(b_q.T).astype(np.float32)  # [128, H_CORE]

    al = np.asarray(alibi, np.float32).reshape(c.B, H, c.S)[:, heads]  # [B,HC,S]
    slope = al[:, :, 1] - al[:, :, 0]                   # [B, H_CORE]
    assert np.allclose(slope[0], slope[1]), "alibi slopes differ across batch"
    # per-q stabilizer ramp for steep slots 0,1: -slope*(0..511), broadcast
    # over partitions; bf16 (per-column rounding cancels in softmax norm)
    ramp = np.arange(512, dtype=np.float32)
    rq = (-slope[0][:2, None] * ramp[None, :]).reshape(1, -1)  # [1, 1024]
    rampq = np.ascontiguousarray(
        np.broadcast_to(rq, (128, 2 * 512))
    ).astype(BF)
    # exp bias per (b, hl, kt, cq): alibi[k] - alibi[cq*512]
    kpos = (np.arange(c.NKT)[:, None] * 128 + np.arange(128)[None, :])  # [NKT,128]
    bias_kq = (
        al[:, :, kpos.reshape(-1)].reshape(c.B, c.H_CORE, c.NKT, 128)[
            :, :, :, None, :
        ]
        - al[:, :, ::512][:, :, None, :, None]          # [B,HC,1,CHQ,1]
    )                                                    # [B,HC,NKT,CHQ,128]
    bias_kq = np.ascontiguousarray(
        bias_kq.transpose(4, 0, 1, 2, 3).reshape(128, -1)
    ).astype(np.float32)

    wd_c = np.asarray(Wd, np.float32).reshape(H, c.HD, c.HID)[heads]
    wd = np.ascontiguousarray(wd_c.reshape(c.MC * 128, c.HID)).astype(BF)

    return {
        "wqkv_qk": wqkv_qk,
        "wqkv_v": wqkv_v,
        "bias_q": bias_q,
        "rampq": rampq,
        "bias_kq": bias_kq,
        "wd": wd,
    }


def build_nc(cfg, debug=False):
    nc = bacc.Bacc("TRN2", target_bir_lowering=False, debug=debug)
    ins = {
        n: nc.dram_tensor(n, sh, dt, kind="ExternalInput").ap()
        for n, (sh, dt) in input_specs(cfg).items()
    }
    outs = {
        n: nc.dram_tensor(n, sh, dt, kind="ExternalOutput").ap()
        for n, (sh, dt) in output_specs(cfg).items()
    }
    with tile.TileContext(nc) as tc:
        with ExitStack() as es:
            build(es, tc, outs, ins, cfg)
    nc.compile()
    return nc


_NC_CACHE = {}


def _get_nc(cfg):
    if cfg not in _NC_CACHE:
        _NC_CACHE[cfg] = build_nc(cfg)
    return _NC_CACHE[cfg]


def _run(inputs, trace=False, **kwargs):
    cfg = FULL
    c = cfg
    hidden_states = np.asarray(inputs["hidden_states"], np.float32)
    residual = np.asarray(inputs["residual"], np.float32)
    alibi = np.asarray(inputs["alibi"], np.float32)
    Wqkv = np.asarray(inputs["Wqkv"], np.float32)
    bqkv = np.asarray(inputs["bqkv"], np.float32)
    Wd = np.asarray(inputs["Wd"], np.float32)
    bd = np.asarray(inputs["bd"], np.float32)

    nc = _get_nc(cfg)
    hs_pack = prep_shared(hidden_states, cfg)
    in_maps = []
    for core in range(N_CORES):
        # stride-8 head assignment: slot j gets head core + 8j, so each
        # slot's slope range is tight and the tile-skip pattern (which must
        # be uniform across the shared NEFF) stays safe and effective
        heads = [core + 8 * j for j in range(c.H_CORE)]
        m = {"hs_pack": hs_pack}
        m.update(prep_core(alibi, Wqkv, bqkv, Wd, heads, cfg))
        in_maps.append(m)

    res = run_bass_kernel_spmd(
        nc, in_maps, core_ids=list(range(N_CORES)), trace=trace, **kwargs
    )
    acc = np.zeros((c.TOKS, c.HID), np.float64)
    for r in res.results:
        acc += r["out_part"].astype(np.float64)
    # V bias folds through the dense layer: ctx+bv -> +bv@Wd on every row
    bv = np.asarray(bqkv, np.float64).reshape(-1, 3, c.HD)[:, 2, :].reshape(-1)
    bvwd = bv @ np.asarray(Wd, np.float64)
    out = (
        acc.reshape(c.B, c.S, c.HID)
        + residual.astype(np.float64) + bd + bvwd
    )
    return out.astype(np.float32), res


def kernel(**inputs):
    out, _ = _run(inputs, trace=False)
    return out


# revision 26
# speedup vs baseline: 1.1137x; 1.0101x over previous
"""BLOOM attention block on 8 TRN2 NeuronCores.

Tensor-parallel over heads: core c computes heads 4c..4c+3 for both batches.
Device math in bf16 with fp32 accumulation. v3 design (v2 + PE-cycle cuts):

  All weights (Wqkv-qk stripes, Wqkv-v, Wd) SBUF-resident; wqk loaded in
  kt-group chunks so the first matmuls start ~8us in (subtile deps).
  Single PSUM pool with 8 bank tags A0..A7 shared across phases so bank
  reuse is tracked per-tag (no phase-boundary PSUM drain stalls).
  Per batch half b (2048 tokens):
    phase 1a: Q^T/K^T kt-outer streaming - 8 PSUM accumulators; hs tiles
              [128,512] streamed from a host-packed layout. Q stripes get
              bias on DVE evac; K stripes are plain ACT copies (the K bias
              adds q.bk to every score of a query - constant per softmax
              row, so it cancels and is dropped).
    phase 1b: V = hs Wv, 4 PSUM accumulators per 512-token group; plain
              ACT copy evac (V bias folds into a host-side bv@Wd row).
    phase 2:  per (cq, head): causal-tiled transposed scores; NO alibi
              rank-1 matmul: the per-column stabilizer ramp cancels in
              softmax normalization, so it is only needed to keep exp in
              fp32/bf16 range - slots 0,1 (steep slopes) get it as a DVE
              add of a broadcast ramp tile; slots 2,3 skip it entirely
              (max exp arg ~ e^39, in range). Diagonal-square mask added
              by VE on [128,128]; exp on ACT straight from PSUM with
              per-partition bias alibi[k]-alibi[cq*512]; ones-reduce and
              ctx^T=V^T P restricted to the live column range.
    phase 3:  dense out_part = ctx Wd (bf16 partials), emission deferred
              one q-chunk so the softmax-normalize tail hides under
              attention.

Host: shards/casts inputs, then
  out = residual + bd + bv@Wd + sum_c out_part_c.
Self-contained: shapes hardcoded for B=2, S=2048, HID=4096, H=32, 8 cores.
"""

import math
from contextlib import ExitStack
from dataclasses import dataclass

import ml_dtypes
import numpy as np

import concourse.bacc as bacc
import concourse.mybir as mybir
import concourse.tile as tile
from concourse.bass import ts
from concourse.bass_utils import run_bass_kernel_spmd

F32 = mybir.dt.float32
BF16 = mybir.dt.bfloat16
AF = mybir.ActivationFunctionType
ALU = mybir.AluOpType
BF = ml_dtypes.bfloat16

N_CORES = 8


@dataclass(frozen=True)
class Cfg:
    B: int = 2
    S: int = 2048
    HID: int = 4096
    H_CORE: int = 4          # heads handled by this core
    HD: int = 128

    @property
    def TOKS(self):
        return self.B * self.S

    @property
    def KT(self):
        return self.HID // 128          # hid tiles (contraction)

    @property
    def QK_CT(self):
        return 2 * self.H_CORE          # q+k coltiles

    @property
    def VC(self):
        return self.H_CORE * self.HD    # v columns (<= 512)

    @property
    def NKT(self):
        return self.S // 128            # k tiles per sequence (per b)

    @property
    def MC(self):
        return self.VC // 128           # dense contraction chunks

    @property
    def CHQ(self):
        return self.S // 512            # q chunks per sequence (per b)

    @property
    def NG(self):
        return self.S // 512            # phase-1 groups per b


FULL = Cfg()

# qk stripe ct -> PSUM bank tag. Interleaved so banks A0..A3 (reused by the
# V accumulators and later by ctx/dense) are freed by alternating DVE/ACT
# evacs and are ready first.
QK_BANK = {0: 0, 4: 1, 1: 2, 5: 3, 2: 4, 6: 5, 3: 6, 7: 7}
QK_EVAC_ORDER = [0, 4, 1, 5, 2, 6, 3, 7]


def keep_tile(slot, kt, cq):
    """Alibi-decay tile skip: heads are sharded stride-8, so slot j's
    shallowest slope is 2^-(2j+2); a k-tile whose closest (k,q) pair is
    dist away contributes < e^-(slope*dist) relative - drop below e^-12."""
    dd = kt - 4 * cq
    if dd >= 0:
        return True
    slope_min = 2.0 ** (-2 * (slot + 1))
    dist = (4 * cq - kt) * 128 - 127
    return slope_min * dist <= 12.0


def input_specs(cfg: Cfg):
    c = cfg
    return {
        # host-packed hs: per (b,g): [128, KT*512] contiguous per partition
        "hs_pack": ([c.B * c.NG, 128, c.KT * 512], BF16),
        "wqkv_qk": ([c.QK_CT, 128, c.HID], BF16),
        "wqkv_v": ([128, c.KT * c.VC], BF16),
        "bias_q": ([128, c.H_CORE], F32),
        # per-column stabilizer ramp for steep slots 0,1: -slope*(0..511),
        # broadcast over partitions (bf16 rounding is a per-column factor
        # that cancels in softmax normalization)
        "rampq": ([128, 2 * 512], BF16),
        # exp bias per (b, hl, kt, cq): alibi[k] - alibi[cq*512]
        "bias_kq": ([128, c.B * c.H_CORE * c.NKT * c.CHQ], F32),
        "wd": ([c.MC * 128, c.HID], BF16),
    }


def output_specs(cfg: Cfg):
    return {"out_part": ([cfg.TOKS, cfg.HID], BF16)}


def build(ctx: ExitStack, tc, outs, ins, cfg: Cfg):
    c = cfg
    nc = tc.nc
    hs_pack = ins["hs_pack"]
    wqkv_qk, wqkv_v, wd = ins["wqkv_qk"], ins["wqkv_v"], ins["wd"]
    bias_q, rampq, bias_kq = ins["bias_q"], ins["rampq"], ins["bias_kq"]
    out_part = outs["out_part"]

    # ---- persistent SBUF ----
    persist = ctx.enter_context(tc.tile_pool(name="persist", bufs=1))
    wqk_sb = persist.tile([128, c.QK_CT, c.HID], BF16, tag="wqk")
    wv_sb = persist.tile([128, c.KT * c.VC], BF16, tag="wv")
    wd_sb = persist.tile([128, c.MC, c.HID], BF16, tag="wd")
    qkt_sb = persist.tile([128, c.QK_CT, c.S], BF16, tag="qkt")      # per-b
    v_sb = persist.tile([128, c.NKT, c.VC], BF16, tag="v")           # per-b
    bias_q_sb = persist.tile([128, c.H_CORE], F32, tag="bias_q")
    rampq_sb = persist.tile([128, 2, 512], BF16, tag="rampq")
    bias_kq_sb = persist.tile(
        [128, c.B * c.H_CORE * c.NKT * c.CHQ], F32, tag="bias_kq"
    )
    ones_col = persist.tile([128, 1], BF16, tag="ones_col")

    # single PSUM pool: 8 bank tags shared by every phase so reuse is
    # tracked per-bank (fine-grained WAR instead of pool-boundary drains)
    psum = ctx.enter_context(tc.tile_pool(name="psum", bufs=1, space="PSUM"))

    def bank(i, shape=None, name=None):
        return psum.tile(
            shape or [128, 512], F32, tag=f"A{i}", name=name or f"A{i}"
        )

    # ---- startup DMA order: first hs chunk, tiny aux, then wqk kt-group 0.
    # Phase 1a's chunk loop has ~5.5us/chunk of DMA slack (6.9us compute vs
    # 1.4us hs traffic), so the remaining wqk kt-groups, wv and wd stream in
    # one kt-group ahead of use, doled out inside the loop.
    hs_pool = ctx.enter_context(tc.tile_pool(name="hs", bufs=4))
    pre_hs = {}
    for ch in range(3):
        t = hs_pool.tile([128, 2, 512], BF16, tag="hs", name="hs_t")
        nc.sync.dma_start(out=t[:], in_=hs_pack[0][:, ts(ch, 2 * 512)])
        pre_hs[ch] = t
    for ct in range(c.QK_CT):
        nc.sync.dma_start(
            out=wqk_sb[:, ct, ts(0, 1024)], in_=wqkv_qk[ct][:, ts(0, 1024)]
        )
    nc.sync.dma_start(out=bias_q_sb[:], in_=bias_q[:])
    nc.sync.dma_start(out=rampq_sb[:], in_=rampq[:])
    nc.sync.dma_start(out=bias_kq_sb[:], in_=bias_kq[:])
    # deferred weight loads: [chunk index in b0's 1a when to issue] -> DMAs
    # (each kt-group split across two chunk slots to interleave with hs)
    wload = {}
    for kg in range(1, c.KT // 8):
        for half in range(2):
            wload.setdefault(4 * (kg - 1) + half, []).extend(
                (lambda kg=kg, ct=ct: nc.sync.dma_start(
                    out=wqk_sb[:, ct, ts(kg, 1024)],
                    in_=wqkv_qk[ct][:, ts(kg, 1024)],
                ))
                for ct in range(4 * half, 4 * half + 4)
            )
    for wc in range(8):
        wload.setdefault(12 + wc, []).append(
            lambda wc=wc: nc.sync.dma_start(
                out=wv_sb[:, ts(wc, 4 * c.VC)],
                in_=wqkv_v[:, ts(wc, 4 * c.VC)],
            )
        )
    for mc in range(c.MC):
        wload.setdefault(20 + mc, []).append(
            lambda mc=mc: nc.sync.dma_start(
                out=wd_sb[:, mc, :], in_=wd[ts(mc, 128), :]
            )
        )
    nc.gpsimd.memset(ones_col[:], 1.0)
    # no-op select to pull the gpsimd affine_select ucode library in now,
    # hidden under the startup DMAs (first real use would stall ~6us)
    nc.gpsimd.affine_select(
        out=ones_col[:], in_=ones_col[:],
        compare_op=ALU.is_ge, fill=1.0,
        base=0, pattern=[[1, 1]], channel_multiplier=0,
    )

    for b in range(c.B):
        # ================= Phase 1a: Q^T/K^T =================
        _sid1, _ = nc.enter_named_scope(f"p1qk_b{b}", False)
        for g in range(c.NG):
            qk_ps = {
                ct: bank(QK_BANK[ct], name=f"qk_ps{ct}")
                for ct in range(c.QK_CT)
            }
            for ch in range(c.KT // 2):
                if b == 0 and g == 0 and ch in pre_hs:
                    hs_t = pre_hs.pop(ch)
                else:
                    hs_t = hs_pool.tile(
                        [128, 2, 512], BF16, tag="hs", name="hs_t"
                    )
                    nc.sync.dma_start(
                        out=hs_t[:],
                        in_=hs_pack[b * c.NG + g][:, ts(ch, 2 * 512)],
                    )
                if b == 0:
                    for w in wload.pop(g * (c.KT // 2) + ch, []):
                        w()
                for k4 in range(2):
                    kt = ch * 2 + k4
                    for ct in range(c.QK_CT):
                        nc.tensor.matmul(
                            qk_ps[ct][:],
                            wqk_sb[:, ct, ts(kt, 128)],
                            hs_t[:, k4, :],
                            start=(kt == 0), stop=(kt == c.KT - 1),
                        )
            for ct in QK_EVAC_ORDER:
                if ct < c.H_CORE:
                    # Q stripe: add bias on DVE
                    nc.vector.tensor_scalar(
                        qkt_sb[:, ct, ts(g, 512)], qk_ps[ct][:],
                        bias_q_sb[:, ct:ct + 1], None, ALU.add,
                    )
                else:
                    # K stripe: bias cancels in softmax - plain ACT copy
                    nc.scalar.copy(qkt_sb[:, ct, ts(g, 512)], qk_ps[ct][:])
        nc.leave_named_scope(f"p1qk_b{b}", _sid1, False)

        # ================= Phase 1b: V =================
        _sid2, _ = nc.enter_named_scope(f"p1v_b{b}", False)
        for g in range(c.NG):
            v_ps = {tt: bank(tt, name=f"v_ps{tt}") for tt in range(4)}
            for ch in range(c.KT // 2):
                hs_t = hs_pool.tile([128, 2, 512], BF16, tag="hs", name="hs_t")
                nc.sync.dma_start(
                    out=hs_t[:],
                    in_=hs_pack[b * c.NG + g][:, ts(ch, 2 * 512)],
                )
                for k4 in range(2):
                    kt = ch * 2 + k4
                    for tt in range(4):
                        nc.tensor.matmul(
                            v_ps[tt][:],
                            hs_t[:, k4, ts(tt, 128)],
                            wv_sb[:, ts(kt, c.VC)],
                            start=(kt == 0), stop=(kt == c.KT - 1),
                        )
            for tt in range(4):
                nc.scalar.copy(v_sb[:, g * 4 + tt, :], v_ps[tt][:])
        nc.leave_named_scope(f"p1v_b{b}", _sid2, False)

        # ============ Phase 2+3: attention fused with dense ============
        _sid3, _ = nc.enter_named_scope(f"p23_b{b}", False)
        with (
            tc.tile_pool(name=f"a_pt{b}", bufs=1) as pt_pool,
            tc.tile_pool(name=f"a_roll{b}", bufs=1) as roll_pool,
            tc.tile_pool(name=f"a_sm{b}", bufs=1) as sm_pool,
            tc.tile_pool(name=f"d_out{b}", bufs=1) as o_pool,
        ):
            # manual bank rotation (replaces per-phase PSUM pools)
            s_rot = [4, 5, 6]
            ctx_rot = [0, 1]
            d_rot = [2, 3]
            rr = {"s": 0, "c": 0, "d": 0}

            def emit_dense_unit(cq, ctx_roll, sub, nb):
                d_ps = bank(d_rot[rr["d"] % 2], name="d_ps")
                rr["d"] += 1
                for mc in range(c.MC):
                    nc.tensor.matmul(
                        d_ps[:],
                        ctx_roll[:, mc, ts(sub, 128)],
                        wd_sb[:, mc, ts(nb, 512)],
                        start=(mc == 0), stop=(mc == c.MC - 1),
                    )
                o_sb = o_pool.tile([128, 512], BF16, tag="o_sb", bufs=4)
                if (sub + nb) % 2 == 0:
                    nc.scalar.copy(o_sb[:], d_ps[:])
                else:
                    nc.vector.tensor_scalar(
                        o_sb[:], d_ps[:], 0.0, None, ALU.add
                    )
                nc.sync.dma_start(
                    out=out_part[
                        b * c.S + cq * 512 + sub * 128:
                        b * c.S + cq * 512 + (sub + 1) * 128,
                        ts(nb, 512),
                    ],
                    in_=o_sb[:],
                )

            NB = c.HID // 512
            dense_q = []       # deferred dense units: (cq, roll, sub, nb)
            pend = []          # flat software pipeline across heads and cqs
            norm_done = [0] * c.CHQ

            def pop_one(need=c.H_CORE):
                e = pend.pop(0)
                e["b"]()
                if e["norm"] is not None:
                    e["norm"]()
                    norm_done[e["cq"]] += 1
                    # drain up to 8 deferred dense units per finished head
                    for _ in range(8):
                        if dense_q and norm_done[dense_q[0][0]] >= need:
                            emit_dense_unit(*dense_q.pop(0))

            for cq in range(c.CHQ):
                ktmax = 4 * (cq + 1)
                ctx_roll = roll_pool.tile(
                    [128, c.MC, 512], BF16, tag="ctx_roll", bufs=2
                )
                for hl in range(c.H_CORE):
                    qT = qkt_sb[:, hl, :]
                    kT = qkt_sb[:, c.H_CORE + hl, :]
                    sum_ps = bank(7, shape=[1, 512], name="sum_ps")
                    ctx_ps = bank(ctx_rot[rr["c"] % 2], name="ctx_ps")
                    rr["c"] += 1
                    kts = [kt for kt in range(ktmax) if keep_tile(hl, kt, cq)]

                    def stage_a(kt, cq=cq, hl=hl, qT=qT, kT=kT):
                        dd = kt - 4 * cq
                        qlo = max(dd, 0) * 128
                        s_ps = bank(s_rot[rr["s"] % 3], name="s_ps")
                        rr["s"] += 1
                        nc.tensor.matmul(
                            s_ps[:, qlo:512], kT[:, ts(kt, 128)],
                            qT[:, cq * 512 + qlo:cq * 512 + 512],
                            start=True, stop=True,
                        )
                        if hl < 2:
                            # steep slots need the per-q stabilizer ramp for
                            # exp range (it cancels in normalization)
                            nc.vector.tensor_tensor(
                                s_ps[:, qlo:512], s_ps[:, qlo:512],
                                rampq_sb[:, hl, qlo:512], ALU.add,
                            )
                        pt = pt_pool.tile([128, 512], BF16, tag="pt", bufs=3)
                        bidx = ((b * c.H_CORE + hl) * c.NKT + kt) * c.CHQ + cq
                        nc.scalar.activation(
                            pt[:, qlo:512], s_ps[:, qlo:512], AF.Exp,
                            bias=bias_kq_sb[:, bidx:bidx + 1], scale=1.0,
                        )
                        if dd >= 0:
                            # causal mask applied post-exp on the idle gpsimd
                            # engine: zero the k>q triangle of the diagonal
                            # square (overwrites any exp overflow exactly)
                            nc.gpsimd.affine_select(
                                out=pt[:, qlo:qlo + 128],
                                in_=pt[:, qlo:qlo + 128],
                                compare_op=ALU.is_ge, fill=0.0,
                                base=0, pattern=[[1, 128]],
                                channel_multiplier=-1,
                            )
                        return (qlo, pt)

                    def stage_b(kt, qlo, pt, hl=hl, kts=kts,
                                sum_ps=sum_ps, ctx_ps=ctx_ps):
                        st, sp = (kt == kts[0]), (kt == kts[-1])
                        # ctx first: its bank never waits on the normalize
                        # chain, so the PE keeps streaming if sum's does
                        nc.tensor.matmul(
                            ctx_ps[:, qlo:512],
                            v_sb[:, kt, ts(hl, 128)],
                            pt[:, qlo:512],
                            start=st, stop=sp,
                        )
                        nc.tensor.matmul(
                            sum_ps[:, qlo:512], ones_col[:], pt[:, qlo:512],
                            start=st, stop=sp,
                        )

                    def normalize(hl=hl, sum_ps=sum_ps, ctx_ps=ctx_ps,
                                  ctx_roll=ctx_roll):
                        rrow = sm_pool.tile([1, 512], F32, tag="rrow", bufs=1)
                        rrep = sm_pool.tile(
                            [128, 512], F32, tag="rrep", bufs=1
                        )
                        nc.vector.reciprocal_approx_fast(rrow[:], sum_ps[:])
                        nc.gpsimd.partition_broadcast(rrep[:], rrow[:])
                        nc.vector.tensor_tensor(
                            ctx_roll[:, hl, :], ctx_ps[:], rrep[:], ALU.mult,
                        )

                    for kt in kts:
                        qlo, pt = stage_a(kt)
                        pend.append({
                            "b": (lambda kt=kt, qlo=qlo, pt=pt,
                                  sb=stage_b: sb(kt, qlo, pt)),
                            "norm": normalize if kt == kts[-1] else None,
                            "cq": cq,
                        })
                        if len(pend) > 2:
                            pop_one()

                # queue this cq's dense for emission during the next chunk
                dense_q.extend(
                    (cq, ctx_roll, u // NB, u % NB) for u in range(4 * NB)
                )
            while pend:
                # final drain: nothing else fills the PE, so let dense mms
                # queue behind the last normalizes instead of idling
                pop_one(need=c.H_CORE - 1)
            while dense_q:
                emit_dense_unit(*dense_q.pop(0))
        nc.leave_named_scope(f"p23_b{b}", _sid3, False)


# ================= host side =================

def prep_shared(hidden_states, cfg):
    """hs_pack [B*NG, 128, KT*512] bf16 - shared across cores.

    pack[b*NG+g, p, kt*512+tq] = hs[b, g*512+tq, kt*128+p]
    (32 KB contiguous per partition per group -> efficient chunked DMA)
    """
    c = cfg
    hs = np.asarray(hidden_states, np.float32).reshape(c.B, c.NG, 512, c.KT, 128)
    pk = np.ascontiguousarray(hs.transpose(0, 1, 4, 3, 2))
    return pk.reshape(c.B * c.NG, 128, c.KT * 512).astype(BF)


def prep_core(alibi, Wqkv, bqkv, Wd, heads, cfg):
    """Per-core inputs for `heads` (list of H_CORE global head indices)."""
    c = cfg
    inv = 1.0 / math.sqrt(c.HD)
    Wq = np.asarray(Wqkv, np.float32).reshape(c.HID, -1, 3, c.HD)
    bq = np.asarray(bqkv, np.float32).reshape(-1, 3, c.HD)
    H = Wq.shape[1]

    # q cols pre-scaled by inv_norm; ct order: q heads then k heads
    w_q = Wq[:, heads, 0, :] * inv                      # [HID, H_CORE, HD]
    w_k = Wq[:, heads, 1, :]
    w_qk = np.concatenate([w_q, w_k], axis=1)           # [HID, QK_CT, 128]
    # -> [ct][hid_p][kt*128+col]: SBUF stripe rows are hid-within-chunk
    w_qk = w_qk.reshape(c.KT, 128, c.QK_CT, 128).transpose(2, 1, 0, 3)
    wqkv_qk = np.ascontiguousarray(w_qk.reshape(c.QK_CT, 128, c.HID)).astype(BF)

    w_v = Wq[:, heads, 2, :].reshape(c.HID, c.VC)       # [HID, VC]
    w_v = w_v.reshape(c.KT, 128, c.VC).transpose(1, 0, 2)  # [p, kt, vc]
    wqkv_v = np.ascontiguousarray(w_v.reshape(128, c.KT * c.VC)).astype(BF)

    b_q = bq[heads, 0, :] * inv                         # [H_CORE, 128]
    bias_q = np.ascontiguousarray(b_q.T).astype(np.float32)  # [128, H_CORE]

    al = np.asarray(alibi, np.float32).reshape(c.B, H, c.S)[:, heads]  # [B,HC,S]
    slope = al[:, :, 1] - al[:, :, 0]                   # [B, H_CORE]
    assert np.allclose(slope[0], slope[1]), "alibi slopes differ across batch"
    # per-q stabilizer ramp for steep slots 0,1: -slope*(0..511), broadcast
    # over partitions; bf16 (per-column rounding cancels in softmax norm)
    ramp = np.arange(512, dtype=np.float32)
    rq = (-slope[0][:2, None] * ramp[None, :]).reshape(1, -1)  # [1, 1024]
    rampq = np.ascontiguousarray(
        np.broadcast_to(rq, (128, 2 * 512))
    ).astype(BF)
    # exp bias per (b, hl, kt, cq): alibi[k] - alibi[cq*512]
    kpos = (np.arange(c.NKT)[:, None] * 128 + np.arange(128)[None, :])  # [NKT,128]
    bias_kq = (
        al[:, :, kpos.reshape(-1)].reshape(c.B, c.H_CORE, c.NKT, 128)[
            :, :, :, None, :
        ]
        - al[:, :, ::512][:, :, None, :, None]          # [B,HC,1,CHQ,1]
    )                                                    # [B,HC,NKT,CHQ,128]
    bias_kq = np.ascontiguousarray(
        bias_kq.transpose(4, 0, 1, 2, 3).reshape(128, -1)
    ).astype(np.float32)

    wd_c = np.asarray(Wd, np.float32).reshape(H, c.HD, c.HID)[heads]
    wd = np.ascontiguousarray(wd_c.reshape(c.MC * 128, c.HID)).astype(BF)

    return {
        "wqkv_qk": wqkv_qk,
        "wqkv_v": wqkv_v,
        "bias_q": bias_q,
        "rampq": rampq,
        "bias_kq": bias_kq,
        "wd": wd,
    }


def build_nc(cfg, debug=False):
    nc = bacc.Bacc("TRN2", target_bir_lowering=False, debug=debug)
    ins = {
        n: nc.dram_tensor(n, sh, dt, kind="ExternalInput").ap()
        for n, (sh, dt) in input_specs(cfg).items()
    }
    outs = {
        n: nc.dram_tensor(n, sh, dt, kind="ExternalOutput").ap()
        for n, (sh, dt) in output_specs(cfg).items()
    }
    with tile.TileContext(nc) as tc:
        with ExitStack() as es:
            build(es, tc, outs, ins, cfg)
    nc.compile()
    return nc


_NC_CACHE = {}


def _get_nc(cfg):
    if cfg not in _NC_CACHE:
        _NC_CACHE[cfg] = build_nc(cfg)
    return _NC_CACHE[cfg]


def _run(inputs, trace=False, **kwargs):
    cfg = FULL
    c = cfg
    hidden_states = np.asarray(inputs["hidden_states"], np.float32)
    residual = np.asarray(inputs["residual"], np.float32)
    alibi = np.asarray(inputs["alibi"], np.float32)
    Wqkv = np.asarray(inputs["Wqkv"], np.float32)
    bqkv = np.asarray(inputs["bqkv"], np.float32)
    Wd = np.asarray(inputs["Wd"], np.float32)
    bd = np.asarray(inputs["bd"], np.float32)

    nc = _get_nc(cfg)
    hs_pack = prep_shared(hidden_states, cfg)
    in_maps = []
    for core in range(N_CORES):
        # stride-8 head assignment: slot j gets head core + 8j, so each
        # slot's slope range is tight and the tile-skip pattern (which must
        # be uniform across the shared NEFF) stays safe and effective
        heads = [core + 8 * j for j in range(c.H_CORE)]
        m = {"hs_pack": hs_pack}
        m.update(prep_core(alibi, Wqkv, bqkv, Wd, heads, cfg))
        in_maps.append(m)

    res = run_bass_kernel_spmd(
        nc, in_maps, core_ids=list(range(N_CORES)), trace=trace, **kwargs
    )
    acc = np.zeros((c.TOKS, c.HID), np.float64)
    for r in res.results:
        acc += r["out_part"].astype(np.float64)
    # V bias folds through the dense layer: ctx+bv -> +bv@Wd on every row
    bv = np.asarray(bqkv, np.float64).reshape(-1, 3, c.HD)[:, 2, :].reshape(-1)
    bvwd = bv @ np.asarray(Wd, np.float64)
    out = (
        acc.reshape(c.B, c.S, c.HID)
        + residual.astype(np.float64) + bd + bvwd
    )
    return out.astype(np.float32), res


def kernel(**inputs):
    out, _ = _run(inputs, trace=False)
    return out


# revision 31
# speedup vs baseline: 1.1181x; 1.0040x over previous
"""BLOOM attention block on 8 TRN2 NeuronCores.

Tensor-parallel over heads: core c computes heads 4c..4c+3 for both batches.
Device math in bf16 with fp32 accumulation. v3 design (v2 + PE-cycle cuts):

  All weights (Wqkv-qk stripes, Wqkv-v, Wd) SBUF-resident; wqk loaded in
  kt-group chunks so the first matmuls start ~8us in (subtile deps).
  Single PSUM pool with 8 bank tags A0..A7 shared across phases so bank
  reuse is tracked per-tag (no phase-boundary PSUM drain stalls).
  Per batch half b (2048 tokens):
    phase 1a: Q^T/K^T kt-outer streaming - 8 PSUM accumulators; hs tiles
              [128,512] streamed from a host-packed layout. Q stripes get
              bias on DVE evac; K stripes are plain ACT copies (the K bias
              adds q.bk to every score of a query - constant per softmax
              row, so it cancels and is dropped).
    phase 1b: V = hs Wv, 4 PSUM accumulators per 512-token group; plain
              ACT copy evac (V bias folds into a host-side bv@Wd row).
    phase 2:  per (cq, head): causal-tiled transposed scores; NO alibi
              rank-1 matmul: the per-column stabilizer ramp cancels in
              softmax normalization, so it is only needed to keep exp in
              fp32/bf16 range - slots 0,1 (steep slopes) get it as a DVE
              add of a broadcast ramp tile; slots 2,3 skip it entirely
              (max exp arg ~ e^39, in range). Diagonal-square mask added
              by VE on [128,128]; exp on ACT straight from PSUM with
              per-partition bias alibi[k]-alibi[cq*512]; ones-reduce and
              ctx^T=V^T P restricted to the live column range.
    phase 3:  dense out_part = ctx Wd (bf16 partials), emission deferred
              one q-chunk so the softmax-normalize tail hides under
              attention.

Host: shards/casts inputs, then
  out = residual + bd + bv@Wd + sum_c out_part_c.
Self-contained: shapes hardcoded for B=2, S=2048, HID=4096, H=32, 8 cores.
"""

import math
from contextlib import ExitStack
from dataclasses import dataclass

import ml_dtypes
import numpy as np

import concourse.bacc as bacc
import concourse.mybir as mybir
import concourse.tile as tile
from concourse.bass import ts
from concourse.bass_utils import run_bass_kernel_spmd

F32 = mybir.dt.float32
BF16 = mybir.dt.bfloat16
AF = mybir.ActivationFunctionType
ALU = mybir.AluOpType
BF = ml_dtypes.bfloat16

N_CORES = 8


@dataclass(frozen=True)
class Cfg:
    B: int = 2
    S: int = 2048
    HID: int = 4096
    H_CORE: int = 4          # heads handled by this core
    HD: int = 128

    @property
    def TOKS(self):
        return self.B * self.S

    @property
    def KT(self):
        return self.HID // 128          # hid tiles (contraction)

    @property
    def QK_CT(self):
        return 2 * self.H_CORE          # q+k coltiles

    @property
    def VC(self):
        return self.H_CORE * self.HD    # v columns (<= 512)

    @property
    def NKT(self):
        return self.S // 128            # k tiles per sequence (per b)

    @property
    def MC(self):
        return self.VC // 128           # dense contraction chunks

    @property
    def CHQ(self):
        return self.S // 512            # q chunks per sequence (per b)

    @property
    def NG(self):
        return self.S // 512            # phase-1 groups per b


FULL = Cfg()

# qk stripe ct -> PSUM bank tag. Interleaved so banks A0..A3 (reused by the
# V accumulators and later by ctx/dense) are freed by alternating DVE/ACT
# evacs and are ready first.
QK_BANK = {0: 0, 4: 1, 1: 2, 5: 3, 2: 4, 6: 5, 3: 6, 7: 7}
QK_EVAC_ORDER = [0, 4, 1, 5, 2, 6, 3, 7]


def keep_tile(slot, kt, cq):
    """Alibi-decay tile skip: heads are sharded stride-8, so slot j's
    shallowest slope is 2^-(2j+2); a k-tile whose closest (k,q) pair is
    dist away contributes < e^-(slope*dist) relative - drop below e^-12."""
    dd = kt - 4 * cq
    if dd >= 0:
        return True
    slope_min = 2.0 ** (-2 * (slot + 1))
    dist = (4 * cq - kt) * 128 - 127
    return slope_min * dist <= 12.0


def input_specs(cfg: Cfg):
    c = cfg
    return {
        # host-packed hs: per (b,g): [128, KT*512] contiguous per partition
        "hs_pack": ([c.B * c.NG, 128, c.KT * 512], BF16),
        "wqkv_qk": ([c.QK_CT, 128, c.HID], BF16),
        "wqkv_v": ([128, c.KT * c.VC], BF16),
        "bias_q": ([128, c.H_CORE], F32),
        # per-column stabilizer ramp for steep slots 0,1: -slope*(0..511),
        # broadcast over partitions (bf16 rounding is a per-column factor
        # that cancels in softmax normalization)
        "rampq": ([128, 2 * 512], BF16),
        # exp bias per (b, hl, kt, cq): alibi[k] - alibi[cq*512]
        "bias_kq": ([128, c.B * c.H_CORE * c.NKT * c.CHQ], F32),
        "wd": ([c.MC * 128, c.HID], BF16),
    }


def output_specs(cfg: Cfg):
    return {"out_part": ([cfg.TOKS, cfg.HID], BF16)}


def build(ctx: ExitStack, tc, outs, ins, cfg: Cfg):
    c = cfg
    nc = tc.nc
    hs_pack = ins["hs_pack"]
    wqkv_qk, wqkv_v, wd = ins["wqkv_qk"], ins["wqkv_v"], ins["wd"]
    bias_q, rampq, bias_kq = ins["bias_q"], ins["rampq"], ins["bias_kq"]
    out_part = outs["out_part"]

    # ---- persistent SBUF ----
    persist = ctx.enter_context(tc.tile_pool(name="persist", bufs=1))
    wqk_sb = persist.tile([128, c.QK_CT, c.HID], BF16, tag="wqk")
    wv_sb = persist.tile([128, c.KT * c.VC], BF16, tag="wv")
    wd_sb = persist.tile([128, c.MC, c.HID], BF16, tag="wd")
    qkt_sb = persist.tile([128, c.QK_CT, c.S], BF16, tag="qkt")      # per-b
    v_sb = persist.tile([128, c.NKT, c.VC], BF16, tag="v")           # per-b
    bias_q_sb = persist.tile([128, c.H_CORE], F32, tag="bias_q")
    rampq_sb = persist.tile([128, 2, 512], BF16, tag="rampq")
    bias_kq_sb = persist.tile(
        [128, c.B * c.H_CORE * c.NKT * c.CHQ], F32, tag="bias_kq"
    )
    ones_col = persist.tile([128, 1], BF16, tag="ones_col")
    warm = persist.tile([128, 8], F32, tag="warm")

    # single PSUM pool: 8 bank tags shared by every phase so reuse is
    # tracked per-bank (fine-grained WAR instead of pool-boundary drains)
    psum = ctx.enter_context(tc.tile_pool(name="psum", bufs=1, space="PSUM"))

    def bank(i, shape=None, name=None):
        return psum.tile(
            shape or [128, 512], F32, tag=f"A{i}", name=name or f"A{i}"
        )

    # ---- startup DMA order: first hs chunk, tiny aux, then wqk kt-group 0.
    # Phase 1a's chunk loop has ~5.5us/chunk of DMA slack (6.9us compute vs
    # 1.4us hs traffic), so the remaining wqk kt-groups, wv and wd stream in
    # one kt-group ahead of use, doled out inside the loop.
    hs_pool = ctx.enter_context(tc.tile_pool(name="hs", bufs=4))
    pre_hs = {}
    for ch in range(3):
        t = hs_pool.tile([128, 2, 512], BF16, tag="hs", name="hs_t")
        nc.sync.dma_start(out=t[:], in_=hs_pack[0][:, ts(ch, 2 * 512)])
        pre_hs[ch] = t
    for ct in range(c.QK_CT):
        nc.sync.dma_start(
            out=wqk_sb[:, ct, ts(0, 1024)], in_=wqkv_qk[ct][:, ts(0, 1024)]
        )
    nc.sync.dma_start(out=bias_q_sb[:], in_=bias_q[:])
    nc.sync.dma_start(out=rampq_sb[:], in_=rampq[:])
    nc.sync.dma_start(out=bias_kq_sb[:], in_=bias_kq[:])
    # deferred weight loads: [chunk index in b0's 1a when to issue] -> DMAs
    # (each kt-group split across two chunk slots to interleave with hs)
    wload = {}
    for kg in range(1, c.KT // 8):
        for half in range(2):
            wload.setdefault(4 * (kg - 1) + half, []).extend(
                (lambda kg=kg, ct=ct: nc.sync.dma_start(
                    out=wqk_sb[:, ct, ts(kg, 1024)],
                    in_=wqkv_qk[ct][:, ts(kg, 1024)],
                ))
                for ct in range(4 * half, 4 * half + 4)
            )
    for wc in range(8):
        wload.setdefault(12 + wc, []).append(
            lambda wc=wc: nc.sync.dma_start(
                out=wv_sb[:, ts(wc, 4 * c.VC)],
                in_=wqkv_v[:, ts(wc, 4 * c.VC)],
            )
        )
    for mc in range(c.MC):
        wload.setdefault(20 + mc, []).append(
            lambda mc=mc: nc.sync.dma_start(
                out=wd_sb[:, mc, :], in_=wd[ts(mc, 128), :]
            )
        )
    nc.gpsimd.memset(ones_col[:], 1.0)
    nc.gpsimd.memset(warm[:], 0.0)
    # dummy broadcast to pull in the gpsimd ucode library (and pay its
    # ~6us first-call IRAM load) now, hidden under the startup DMAs -
    # the first real partition_broadcast sits in the normalize chain
    nc.gpsimd.partition_broadcast(warm[:], warm[0:1, :])

    for b in range(c.B):
        # ================= Phase 1a: Q^T/K^T =================
        _sid1, _ = nc.enter_named_scope(f"p1qk_b{b}", False)
        for g in range(c.NG):
            qk_ps = {
                ct: bank(QK_BANK[ct], name=f"qk_ps{ct}")
                for ct in range(c.QK_CT)
            }
            for ch in range(c.KT // 2):
                if b == 0 and g == 0 and ch in pre_hs:
                    hs_t = pre_hs.pop(ch)
                else:
                    hs_t = hs_pool.tile(
                        [128, 2, 512], BF16, tag="hs", name="hs_t"
                    )
                    nc.sync.dma_start(
                        out=hs_t[:],
                        in_=hs_pack[b * c.NG + g][:, ts(ch, 2 * 512)],
                    )
                if b == 0:
                    for w in wload.pop(g * (c.KT // 2) + ch, []):
                        w()
                for k4 in range(2):
                    kt = ch * 2 + k4
                    for ct in range(c.QK_CT):
                        nc.tensor.matmul(
                            qk_ps[ct][:],
                            wqk_sb[:, ct, ts(kt, 128)],
                            hs_t[:, k4, :],
                            start=(kt == 0), stop=(kt == c.KT - 1),
                        )
            for ct in QK_EVAC_ORDER:
                if ct < c.H_CORE:
                    # Q stripe: add bias on DVE
                    nc.vector.tensor_scalar(
                        qkt_sb[:, ct, ts(g, 512)], qk_ps[ct][:],
                        bias_q_sb[:, ct:ct + 1], None, ALU.add,
                    )
                else:
                    # K stripe: bias cancels in softmax - plain ACT copy
                    nc.scalar.copy(qkt_sb[:, ct, ts(g, 512)], qk_ps[ct][:])
        nc.leave_named_scope(f"p1qk_b{b}", _sid1, False)

        # ================= Phase 1b: V =================
        _sid2, _ = nc.enter_named_scope(f"p1v_b{b}", False)
        for g in range(c.NG):
            v_ps = {tt: bank(tt, name=f"v_ps{tt}") for tt in range(4)}
            for ch in range(c.KT // 2):
                hs_t = hs_pool.tile([128, 2, 512], BF16, tag="hs", name="hs_t")
                nc.sync.dma_start(
                    out=hs_t[:],
                    in_=hs_pack[b * c.NG + g][:, ts(ch, 2 * 512)],
                )
                for k4 in range(2):
                    kt = ch * 2 + k4
                    for tt in range(4):
                        nc.tensor.matmul(
                            v_ps[tt][:],
                            hs_t[:, k4, ts(tt, 128)],
                            wv_sb[:, ts(kt, c.VC)],
                            start=(kt == 0), stop=(kt == c.KT - 1),
                        )
            for tt in range(4):
                nc.scalar.copy(v_sb[:, g * 4 + tt, :], v_ps[tt][:])
        nc.leave_named_scope(f"p1v_b{b}", _sid2, False)

        # ============ Phase 2+3: attention fused with dense ============
        _sid3, _ = nc.enter_named_scope(f"p23_b{b}", False)
        with (
            tc.tile_pool(name=f"a_pt{b}", bufs=1) as pt_pool,
            tc.tile_pool(name=f"a_roll{b}", bufs=1) as roll_pool,
            tc.tile_pool(name=f"a_sm{b}", bufs=1) as sm_pool,
            tc.tile_pool(name=f"d_out{b}", bufs=1) as o_pool,
        ):
            # manual bank rotation (replaces per-phase PSUM pools)
            s_rot = [4, 5, 6]
            ctx_rot = [0, 1]
            d_rot = [2, 3]
            rr = {"s": 0, "c": 0, "d": 0}

            def emit_dense_unit(cq, ctx_roll, sub, nb):
                d_ps = bank(d_rot[rr["d"] % 2], name="d_ps")
                rr["d"] += 1
                for mc in range(c.MC):
                    nc.tensor.matmul(
                        d_ps[:],
                        ctx_roll[:, mc, ts(sub, 128)],
                        wd_sb[:, mc, ts(nb, 512)],
                        start=(mc == 0), stop=(mc == c.MC - 1),
                    )
                o_sb = o_pool.tile([128, 512], BF16, tag="o_sb", bufs=4)
                if (sub + nb) % 2 == 0:
                    nc.scalar.copy(o_sb[:], d_ps[:])
                else:
                    nc.vector.tensor_scalar(
                        o_sb[:], d_ps[:], 0.0, None, ALU.add
                    )
                nc.sync.dma_start(
                    out=out_part[
                        b * c.S + cq * 512 + sub * 128:
                        b * c.S + cq * 512 + (sub + 1) * 128,
                        ts(nb, 512),
                    ],
                    in_=o_sb[:],
                )

            NB = c.HID // 512
            dense_q = []       # deferred dense units: (cq, roll, sub, nb)
            pend = []          # flat software pipeline across heads and cqs
            norm_done = [0] * c.CHQ

            def pop_one(need=c.H_CORE):
                e = pend.pop(0)
                e["b"]()
                if e["norm"] is not None:
                    e["norm"]()
                    norm_done[e["cq"]] += 1
                    # drain up to 8 deferred dense units per finished head
                    for _ in range(8):
                        if dense_q and norm_done[dense_q[0][0]] >= need:
                            emit_dense_unit(*dense_q.pop(0))

            for cq in range(c.CHQ):
                ktmax = 4 * (cq + 1)
                ctx_roll = roll_pool.tile(
                    [128, c.MC, 512], BF16, tag="ctx_roll", bufs=2
                )
                for hl in range(c.H_CORE):
                    qT = qkt_sb[:, hl, :]
                    kT = qkt_sb[:, c.H_CORE + hl, :]
                    sum_ps = bank(7, shape=[1, 512], name="sum_ps")
                    ctx_ps = bank(ctx_rot[rr["c"] % 2], name="ctx_ps")
                    rr["c"] += 1
                    kts = [kt for kt in range(ktmax) if keep_tile(hl, kt, cq)]

                    def stage_a(kt, cq=cq, hl=hl, qT=qT, kT=kT):
                        dd = kt - 4 * cq
                        qlo = max(dd, 0) * 128
                        s_ps = bank(s_rot[rr["s"] % 3], name="s_ps")
                        rr["s"] += 1
                        nc.tensor.matmul(
                            s_ps[:, qlo:512], kT[:, ts(kt, 128)],
                            qT[:, cq * 512 + qlo:cq * 512 + 512],
                            start=True, stop=True,
                        )
                        if hl < 2:
                            # steep slots need the per-q stabilizer ramp for
                            # exp range (it cancels in normalization)
                            nc.vector.tensor_tensor(
                                s_ps[:, qlo:512], s_ps[:, qlo:512],
                                rampq_sb[:, hl, qlo:512], ALU.add,
                            )
                        pt = pt_pool.tile([128, 512], BF16, tag="pt", bufs=3)
                        bidx = ((b * c.H_CORE + hl) * c.NKT + kt) * c.CHQ + cq
                        nc.scalar.activation(
                            pt[:, qlo:512], s_ps[:, qlo:512], AF.Exp,
                            bias=bias_kq_sb[:, bidx:bidx + 1], scale=1.0,
                        )
                        if dd >= 0:
                            # causal mask applied post-exp on the idle gpsimd
                            # engine: zero the k>q triangle of the diagonal
                            # square (overwrites any exp overflow exactly)
                            nc.gpsimd.affine_select(
                                out=pt[:, qlo:qlo + 128],
                                in_=pt[:, qlo:qlo + 128],
                                compare_op=ALU.is_ge, fill=0.0,
                                base=0, pattern=[[1, 128]],
                                channel_multiplier=-1,
                            )
                        return (qlo, pt)

                    def stage_b(kt, qlo, pt, hl=hl, kts=kts,
                                sum_ps=sum_ps, ctx_ps=ctx_ps):
                        st, sp = (kt == kts[0]), (kt == kts[-1])
                        # ctx first: its bank never waits on the normalize
                        # chain, so the PE keeps streaming if sum's does
                        nc.tensor.matmul(
                            ctx_ps[:, qlo:512],
                            v_sb[:, kt, ts(hl, 128)],
                            pt[:, qlo:512],
                            start=st, stop=sp,
                        )
                        nc.tensor.matmul(
                            sum_ps[:, qlo:512], ones_col[:], pt[:, qlo:512],
                            start=st, stop=sp,
                        )

                    def normalize(hl=hl, sum_ps=sum_ps, ctx_ps=ctx_ps,
                                  ctx_roll=ctx_roll):
                        rrow = sm_pool.tile([1, 512], F32, tag="rrow", bufs=1)
                        rrep = sm_pool.tile(
                            [128, 512], F32, tag="rrep", bufs=1
                        )
                        nc.vector.reciprocal_approx_fast(rrow[:], sum_ps[:])
                        nc.gpsimd.partition_broadcast(rrep[:], rrow[:])
                        nc.vector.tensor_tensor(
                            ctx_roll[:, hl, :], ctx_ps[:], rrep[:], ALU.mult,
                        )

                    for kt in kts:
                        qlo, pt = stage_a(kt)
                        pend.append({
                            "b": (lambda kt=kt, qlo=qlo, pt=pt,
                                  sb=stage_b: sb(kt, qlo, pt)),
                            "norm": normalize if kt == kts[-1] else None,
                            "cq": cq,
                        })
                        if len(pend) > 2:
                            pop_one()

                # queue this cq's dense for emission during the next chunk
                dense_q.extend(
                    (cq, ctx_roll, u // NB, u % NB) for u in range(4 * NB)
                )
            while pend:
                # final drain: nothing else fills the PE, so let dense mms
                # queue behind the last normalizes instead of idling
                pop_one(need=c.H_CORE - 1)
            while dense_q:
                emit_dense_unit(*dense_q.pop(0))
        nc.leave_named_scope(f"p23_b{b}", _sid3, False)


# ================= host side =================

def prep_shared(hidden_states, cfg):
    """hs_pack [B*NG, 128, KT*512] bf16 - shared across cores.

    pack[b*NG+g, p, kt*512+tq] = hs[b, g*512+tq, kt*128+p]
    (32 KB contiguous per partition per group -> efficient chunked DMA)
    """
    c = cfg
    hs = np.asarray(hidden_states, np.float32).reshape(c.B, c.NG, 512, c.KT, 128)
    pk = np.ascontiguousarray(hs.transpose(0, 1, 4, 3, 2))
    return pk.reshape(c.B * c.NG, 128, c.KT * 512).astype(BF)


def prep_core(alibi, Wqkv, bqkv, Wd, heads, cfg):
    """Per-core inputs for `heads` (list of H_CORE global head indices)."""
    c = cfg
    inv = 1.0 / math.sqrt(c.HD)
    Wq = np.asarray(Wqkv, np.float32).reshape(c.HID, -1, 3, c.HD)
    bq = np.asarray(bqkv, np.float32).reshape(-1, 3, c.HD)
    H = Wq.shape[1]

    # q cols pre-scaled by inv_norm; ct order: q heads then k heads
    w_q = Wq[:, heads, 0, :] * inv                      # [HID, H_CORE, HD]
    w_k = Wq[:, heads, 1, :]
    w_qk = np.concatenate([w_q, w_k], axis=1)           # [HID, QK_CT, 128]
    # -> [ct][hid_p][kt*128+col]: SBUF stripe rows are hid-within-chunk
    w_qk = w_qk.reshape(c.KT, 128, c.QK_CT, 128).transpose(2, 1, 0, 3)
    wqkv_qk = np.ascontiguousarray(w_qk.reshape(c.QK_CT, 128, c.HID)).astype(BF)

    w_v = Wq[:, heads, 2, :].reshape(c.HID, c.VC)       # [HID, VC]
    w_v = w_v.reshape(c.KT, 128, c.VC).transpose(1, 0, 2)  # [p, kt, vc]
    wqkv_v = np.ascontiguousarray(w_v.reshape(128, c.KT * c.VC)).astype(BF)

    b_q = bq[heads, 0, :] * inv                         # [H_CORE, 128]
    bias_q = np.ascontiguousarray(b_q.T).astype(np.float32)  # [128, H_CORE]

    al = np.asarray(alibi, np.float32).reshape(c.B, H, c.S)[:, heads]  # [B,HC,S]
    slope = al[:, :, 1] - al[:, :, 0]                   # [B, H_CORE]
    assert np.allclose(slope[0], slope[1]), "alibi slopes differ across batch"
    # per-q stabilizer ramp for steep slots 0,1: -slope*(0..511), broadcast
    # over partitions; bf16 (per-column rounding cancels in softmax norm)
    ramp = np.arange(512, dtype=np.float32)
    rq = (-slope[0][:2, None] * ramp[None, :]).reshape(1, -1)  # [1, 1024]
    rampq = np.ascontiguousarray(
        np.broadcast_to(rq, (128, 2 * 512))
    ).astype(BF)
    # exp bias per (b, hl, kt, cq): alibi[k] - alibi[cq*512]
    kpos = (np.arange(c.NKT)[:, None] * 128 + np.arange(128)[None, :])  # [NKT,128]
    bias_kq = (
        al[:, :, kpos.reshape(-1)].reshape(c.B, c.H_CORE, c.NKT, 128)[
            :, :, :, None, :
        ]
        - al[:, :, ::512][:, :, None, :, None]          # [B,HC,1,CHQ,1]
    )                                                    # [B,HC,NKT,CHQ,128]
    bias_kq = np.ascontiguousarray(
        bias_kq.transpose(4, 0, 1, 2, 3).reshape(128, -1)
    ).astype(np.float32)

    wd_c = np.asarray(Wd, np.float32).reshape(H, c.HD, c.HID)[heads]
    wd = np.ascontiguousarray(wd_c.reshape(c.MC * 128, c.HID)).astype(BF)

    return {
        "wqkv_qk": wqkv_qk,
        "wqkv_v": wqkv_v,
        "bias_q": bias_q,
        "rampq": rampq,
        "bias_kq": bias_kq,
        "wd": wd,
    }


def build_nc(cfg, debug=False):
    nc = bacc.Bacc("TRN2", target_bir_lowering=False, debug=debug)
    ins = {
        n: nc.dram_tensor(n, sh, dt, kind="ExternalInput").ap()
        for n, (sh, dt) in input_specs(cfg).items()
    }
    outs = {
        n: nc.dram_tensor(n, sh, dt, kind="ExternalOutput").ap()
        for n, (sh, dt) in output_specs(cfg).items()
    }
    with tile.TileContext(nc) as tc:
        with ExitStack() as es:
            build(es, tc, outs, ins, cfg)
    nc.compile()
    return nc


_NC_CACHE = {}


def _get_nc(cfg):
    if cfg not in _NC_CACHE:
        _NC_CACHE[cfg] = build_nc(cfg)
    return _NC_CACHE[cfg]


def _run(inputs, trace=False, **kwargs):
    cfg = FULL
    c = cfg
    hidden_states = np.asarray(inputs["hidden_states"], np.float32)
    residual = np.asarray(inputs["residual"], np.float32)
    alibi = np.asarray(inputs["alibi"], np.float32)
    Wqkv = np.asarray(inputs["Wqkv"], np.float32)
    bqkv = np.asarray(inputs["bqkv"], np.float32)
    Wd = np.asarray(inputs["Wd"], np.float32)
    bd = np.asarray(inputs["bd"], np.float32)

    nc = _get_nc(cfg)
    hs_pack = prep_shared(hidden_states, cfg)
    in_maps = []
    for core in range(N_CORES):
        # stride-8 head assignment: slot j gets head core + 8j, so each
        # slot's slope range is tight and the tile-skip pattern (which must
        # be uniform across the shared NEFF) stays safe and effective
        heads = [core + 8 * j for j in range(c.H_CORE)]
        m = {"hs_pack": hs_pack}
        m.update(prep_core(alibi, Wqkv, bqkv, Wd, heads, cfg))
        in_maps.append(m)

    res = run_bass_kernel_spmd(
        nc, in_maps, core_ids=list(range(N_CORES)), trace=trace, **kwargs
    )
    acc = np.zeros((c.TOKS, c.HID), np.float64)
    for r in res.results:
        acc += r["out_part"].astype(np.float64)
    # V bias folds through the dense layer: ctx+bv -> +bv@Wd on every row
    bv = np.asarray(bqkv, np.float64).reshape(-1, 3, c.HD)[:, 2, :].reshape(-1)
    bvwd = bv @ np.asarray(Wd, np.float64)
    out = (
        acc.reshape(c.B, c.S, c.HID)
        + residual.astype(np.float64) + bd + bvwd
    )
    return out.astype(np.float32), res


def kernel(**inputs):
    out, _ = _run(inputs, trace=False)
    return out
